# revision 22
# baseline (speedup 1.0000x reference)
"""Distributed GraphSAGE kernel for Trainium2 (8 NeuronCores, Bass/Tile). v2

Takes FULL inputs (same keys as setup_inputs()), shards by graph id across 8
cores, runs a single SPMD Bass program (3 SAGE layers + global mean pool +
linear head) with inter-layer AllGathers, returns the FULL [512, 2] output.

v2 changes vs v1:
  - one-hot scatter masks (iota==dst)*1/deg are PRECOMPUTED ON HOST and
    streamed from DRAM as matmul rhs operands (v1 built them per-subtile on
    DVE: ~3.8ms of vector-engine time, the top bottleneck)
  - aggregation runs on 512-wide supertiles (4 node tiles per PSUM tile) with
    EXACT per-(supertile,chunk) edge sub-tile counts (max over the 8 cores so
    the SPMD program is shared); v1 used a global worst-case te_c budget
    (padding 153k slots/core/layer -> 111k)
  - h^T slabs live in SBUF (no DRAM round trip between layers)
  - graph-pool masks precomputed on host as well
  - per-batch (not per-tile) scheduler barriers

Algorithm per core (nodes sharded by graph; batch sorted so each core owns a
contiguous node range; edges assigned to the core owning their dst):
  - table TBL_l holds m_l = h_{l-1} @ Wl_l for ALL nodes (fp16, allgathered);
    TBL split into 4 row-chunks of 2*n_own rows for int16 dma_gather indices
  - per gather batch (NBS supertiles): 4 dma_gather calls (one per chunk)
    with exact slot counts; per supertile: mask-matmuls accumulate
    (mean_agg @ Wl_l)^T into a [128, 512] PSUM tile, self terms
    Wr_l^T @ h^T accumulate into the 4 column slices
  - relu+bias on ACT -> h_l^T slab (SBUF); m_{l+1} matmul per tile -> slab ->
    AllGather
  - layer 3: bias on DVE, transpose tiles via TensorE, pool with precomputed
    per-graph masks, then Wlin matmul + bias
"""
import sys
import os

sys.path.insert(0, "/opt/trn_rl_repo")

import numpy as np
from contextlib import ExitStack
from dataclasses import dataclass

from concourse import bass, mybir, tile, bacc
from concourse import bass_utils
from concourse.masks import make_identity

P = 128
CH = 4              # table row chunks (int16 index limit)
ST = 4              # node tiles per supertile (512-wide PSUM)
BATCH_ST = 4        # supertiles per gather batch (last batch may be smaller)
F16 = mybir.dt.float16
F32 = mybir.dt.float32
I16 = mybir.dt.int16

ABLATE = frozenset()


@dataclass(frozen=True)
class Cfg:
    n_cores: int
    num_nodes: int
    num_edges: int
    in_feat: int
    hidden: int
    num_graphs: int
    num_classes: int
    n_own: int             # padded nodes per core (multiple of ST*128)
    sub_max: tuple         # flat tuple, sub_max[st*CH+c] subtiles per (st,chunk)
    gpc: int               # graphs per core

    @property
    def nt(self):
        return self.n_own // P

    @property
    def n_st(self):
        return self.nt // ST

    @property
    def batches(self):
        """List of (st0, n_st_in_batch)."""
        out = []
        st = 0
        while st < self.n_st:
            n = min(BATCH_ST, self.n_st - st)
            out.append((st, n))
            st += n
        return out

    @property
    def sub_arr(self):
        return np.asarray(self.sub_max, np.int64).reshape(self.n_st, CH)

    @property
    def tot_sub(self):
        return int(self.sub_arr.sum())


def _layout(cfg: Cfg):
    """Static layout tables shared by host preprocessing and program build.

    Returns:
      gcol   [n_st, CH]: first g_t column (within the layer-global column
             space, order (batch, chunk, st, j)) of each (st, chunk) group
      subid  [n_st, CH]: first mask sub-tile id (order (batch, st, chunk, j))
      call_cols [n_batch, CH]: columns per dma_gather call
      batch_col0 [n_batch]: first global column of each batch
    """
    sub = cfg.sub_arr
    batches = cfg.batches
    n_b = len(batches)
    gcol = np.zeros((cfg.n_st, CH), np.int64)
    subid = np.zeros((cfg.n_st, CH), np.int64)
    call_cols = np.zeros((n_b, CH), np.int64)
    batch_col0 = np.zeros(n_b, np.int64)
    col = 0
    for b, (st0, nb) in enumerate(batches):
        batch_col0[b] = col
        for c in range(CH):
            for s in range(nb):
                st = st0 + s
                gcol[st, c] = col
                col += sub[st, c]
            call_cols[b, c] = int(sub[st0:st0 + nb, c].sum())
    sid = 0
    for st in range(cfg.n_st):
        for c in range(CH):
            subid[st, c] = sid
            sid += sub[st, c]
    return gcol, subid, call_cols, batch_col0


def build_program(cfg: Cfg):
    nc = bacc.Bacc(
        "TRN2",
        target_bir_lowering=False,
        debug=False,
        num_devices=cfg.n_cores,
        num_swdge_queues=1,
    )

    NT = cfg.nt
    HID = cfg.hidden
    INF = cfg.in_feat
    GPC = cfg.gpc
    NC = cfg.n_cores
    CHROWS = 2 * cfg.n_own
    sub = cfg.sub_arr
    batches = cfg.batches
    n_b = len(batches)
    gcol, subid, call_cols, batch_col0 = _layout(cfg)
    TOTCOL = int(sub.sum())
    max_batch_cols = int(max(
        (batch_col0[b + 1] if b + 1 < n_b else TOTCOL) - batch_col0[b]
        for b in range(n_b)
    ))
    max_stc_sub = int(sub.max())

    # ---- I/O -------------------------------------------------------------
    xT_d = nc.dram_tensor("xT", [INF, cfg.n_own], F16, kind="ExternalInput")
    idx_d = nc.dram_tensor("eidx", [P, TOTCOL * 8], I16, kind="ExternalInput")
    dstrel_d = nc.dram_tensor("edstrel", [P, TOTCOL], F32, kind="ExternalInput")
    w_d = nc.dram_tensor("ew", [P, TOTCOL], F32, kind="ExternalInput")
    gmask_d = nc.dram_tensor("gmask", [P, NT * GPC], F16, kind="ExternalInput")
    Wl1_d = nc.dram_tensor("Wl1", [INF, HID], F16, kind="ExternalInput")
    Wr1_d = nc.dram_tensor("Wr1", [INF, HID], F16, kind="ExternalInput")
    Wl2_d = nc.dram_tensor("Wl2", [HID, HID], F16, kind="ExternalInput")
    Wr2_d = nc.dram_tensor("Wr2", [HID, HID], F16, kind="ExternalInput")
    Wl3_d = nc.dram_tensor("Wl3", [HID, HID], F16, kind="ExternalInput")
    Wr3_d = nc.dram_tensor("Wr3", [HID, HID], F16, kind="ExternalInput")
    Wlin_d = nc.dram_tensor("Wlin", [HID, cfg.num_classes], F16, kind="ExternalInput")
    bl1_d = nc.dram_tensor("bl1", [HID, 1], F32, kind="ExternalInput")
    bl2_d = nc.dram_tensor("bl2", [HID, 1], F32, kind="ExternalInput")
    bl3_d = nc.dram_tensor("bl3", [HID, 1], F32, kind="ExternalInput")
    blin_d = nc.dram_tensor("blin", [cfg.num_classes, 1], F32, kind="ExternalInput")
    out_d = nc.dram_tensor("out", [cfg.num_classes, GPC], F32, kind="ExternalOutput")

    rg = [list(range(NC))]

    with tile.TileContext(nc) as tc, ExitStack() as ctx:
        sb = ctx.enter_context(tc.tile_pool(name="sb", bufs=1))
        hpool = ctx.enter_context(tc.tile_pool(name="hp", bufs=2))
        sb2 = ctx.enter_context(tc.tile_pool(name="sb2", bufs=4))
        mpool = ctx.enter_context(tc.tile_pool(name="mp", bufs=8))
        gbuf = ctx.enter_context(tc.tile_pool(name="gbuf", bufs=2))
        ps = ctx.enter_context(tc.tile_pool(name="ps", bufs=2, space="PSUM"))
        ps_m = ctx.enter_context(tc.tile_pool(name="psm", bufs=2, space="PSUM"))
        pool_ps = ctx.enter_context(tc.tile_pool(name="pps", bufs=1, space="PSUM"))
        dram = ctx.enter_context(tc.tile_pool(name="dram", bufs=1, space="DRAM"))

        # ---- static SBUF state ------------------------------------------
        ident16 = sb.tile([P, P], F16)
        make_identity(nc, ident16[:])

        # iota over the 512 supertile columns, fp16 (exact for 0..511)
        iota_i = sb.tile([P, ST * P], mybir.dt.int32)
        nc.gpsimd.iota(iota_i[:], pattern=[[1, ST * P]], base=0,
                       channel_multiplier=0)
        iota_f = sb.tile([P, ST * P], F16)
        nc.vector.tensor_copy(iota_f[:], iota_i[:])

        idx_sb = sb.tile([P, TOTCOL * 8], I16)
        nc.sync.dma_start(idx_sb[:], idx_d[:, :])
        dstrel_sb = sb.tile([P, TOTCOL], F32)
        nc.sync.dma_start(dstrel_sb[:], dstrel_d[:, :])
        w_sb = sb.tile([P, TOTCOL], F32)
        nc.sync.dma_start(w_sb[:], w_d[:, :])
        negw_sb = sb.tile([P, TOTCOL], F32)
        nc.vector.tensor_scalar(
            out=negw_sb[:], in0=w_sb[:], scalar1=-1.0, scalar2=None,
            op0=mybir.AluOpType.mult,
        )
        gmask_sb = sb.tile([P, NT * GPC], F16)
        nc.sync.dma_start(gmask_sb[:], gmask_d[:, :])

        def load_w(d, p_, f_, nm):
            t = sb.tile([p_, f_], F16, name=nm, tag=nm)
            nc.sync.dma_start(t[:], d[:, :])
            return t

        Wl1_sb = load_w(Wl1_d, INF, HID, "wl1s")
        Wr1_sb = load_w(Wr1_d, INF, HID, "wr1s")
        Wl2_sb = load_w(Wl2_d, HID, HID, "wl2s")
        Wr2_sb = load_w(Wr2_d, HID, HID, "wr2s")
        Wl3_sb = load_w(Wl3_d, HID, HID, "wl3s")
        Wr3_sb = load_w(Wr3_d, HID, HID, "wr3s")
        Wlin_sb = load_w(Wlin_d, HID, cfg.num_classes, "wlins")
        bl1_sb = sb.tile([HID, 1], F32)
        nc.sync.dma_start(bl1_sb[:], bl1_d[:, :])
        bl2_sb = sb.tile([HID, 1], F32)
        nc.sync.dma_start(bl2_sb[:], bl2_d[:, :])
        bl3_sb = sb.tile([HID, 1], F32)
        nc.sync.dma_start(bl3_sb[:], bl3_d[:, :])
        blin_sb = sb.tile([cfg.num_classes, 1], F32)
        nc.sync.dma_start(blin_sb[:], blin_d[:, :])

        # xT and the two h^T slabs share one 2-slot tag: ht1 reuses xT's slot
        # once layer 1 (the last xT reader) is done
        xT_sb = hpool.tile([INF, cfg.n_own], F16, tag="hx", name="xT")
        nc.sync.dma_start(xT_sb[:], xT_d[:, :])
        tc.no_sync_barrier()

        # ---- internal DRAM ----------------------------------------------
        slabs = [dram.tile([cfg.n_own, HID], F16, tag=f"slab{l}", name=f"slab{l}")
                 for l in range(3)]
        tbls = [dram.tile([NC * cfg.n_own, HID], F16, tag=f"tbl{l}",
                          name=f"tbl{l}", addr_space="Shared")
                for l in range(3)]

        # h^T slabs stay in SBUF between layers (allocated lazily per layer)
        hts = [None, None]

        # ---- P0: m1 = x @ Wl1 (row-major slab) --------------------------
        for t in range(NT):
            m_ps = ps_m.tile([P, HID], F32, tag="mps")
            nc.tensor.matmul(
                out=m_ps[:], lhsT=xT_sb[:, t * P:(t + 1) * P], rhs=Wl1_sb[:],
                start=True, stop=True,
            )
            m_sb = sb2.tile([P, HID], F16, tag="msb")
            nc.vector.tensor_copy(m_sb[:], m_ps[:])
            nc.sync.dma_start(slabs[0][t * P:(t + 1) * P, :], m_sb[:])

        if "noag" not in ABLATE:
            nc.gpsimd.collective_compute(
                "AllGather", mybir.AluOpType.bypass, replica_groups=rg,
                ins=[slabs[0].opt()], outs=[tbls[0].opt()],
            )
        tc.no_sync_barrier()

        # ---- layers ------------------------------------------------------
        for layer in range(3):
            tbl = tbls[layer]
            Wr_sb = (Wr1_sb, Wr2_sb, Wr3_sb)[layer]
            bl_sb = (bl1_sb, bl2_sb, bl3_sb)[layer]
            Wl_next = (Wl2_sb, Wl3_sb, None)[layer]
            if layer < 2:
                hts[layer] = hpool.tile([HID, cfg.n_own], F16, tag="hx",
                                        name=f"ht{layer}")
            h_prev = xT_sb if layer == 0 else hts[layer - 1]
            h_new = hts[layer] if layer < 2 else None

            if layer == 2:
                poolT_ps = pool_ps.tile([HID, GPC], F32, tag="pool")

            for b, (st0, nb_st) in enumerate(batches):
                bc0 = int(batch_col0[b])
                g_t = gbuf.tile([P, max_batch_cols * P], F16, tag="g")
                rel = 0
                for c in range(CH):
                    ncols = int(call_cols[b, c])
                    if ncols == 0 or "nogather" in ABLATE:
                        rel += ncols
                        continue
                    col0 = bc0 + rel
                    nidx = ncols * P
                    nc.gpsimd.dma_gather(
                        out_ap=g_t[:, rel * P:(rel + ncols) * P].rearrange(
                            "p (t e) -> p t e", e=HID),
                        in_ap=tbl[c * CHROWS:(c + 1) * CHROWS, :],
                        idxs_ap=idx_sb[:, col0 * 8:(col0 + ncols) * 8],
                        num_idxs=nidx,
                        num_idxs_reg=nidx,
                        elem_size=HID,
                        single_packet=False,
                    )
                    rel += ncols

                for s in range(nb_st):
                    st = st0 + s
                    out_ps = ps.tile([HID, ST * P], F32, tag="outT")
                    first = True
                    if "nomaskmm" not in ABLATE:
                        for c in range(CH):
                            nsub = int(sub[st, c])
                            if nsub == 0:
                                continue
                            for j in range(nsub):
                                k = int(subid[st, c]) + j
                                k_rel = int(gcol[st, c]) - bc0 + j
                                mask_t = mpool.tile([P, ST * P], F16, tag="mask")
                                if k % 8 < 3:
                                    # DVE path: (iota==dst)*w in one dual-op
                                    nc.vector.tensor_scalar(
                                        out=mask_t[:], in0=iota_f[:],
                                        scalar1=dstrel_sb[:, k:k + 1],
                                        scalar2=w_sb[:, k:k + 1],
                                        op0=mybir.AluOpType.is_equal,
                                        op1=mybir.AluOpType.mult,
                                    )
                                else:
                                    # ACT path (integer grid): t=|d-iota|,
                                    # then relu(w - w*t) = w*(iota==d)
                                    t_t = mpool.tile([P, ST * P], F16,
                                                     tag="mabs")
                                    nc.scalar.activation(
                                        t_t[:], iota_f[:],
                                        mybir.ActivationFunctionType.Abs,
                                        bias=dstrel_sb[:, k:k + 1],
                                        scale=-1.0,
                                    )
                                    nc.scalar.activation(
                                        mask_t[:], t_t[:],
                                        mybir.ActivationFunctionType.Relu,
                                        bias=w_sb[:, k:k + 1],
                                        scale=negw_sb[:, k:k + 1],
                                    )
                                nc.tensor.matmul(
                                    out=out_ps[:],
                                    lhsT=g_t[:, k_rel * P:(k_rel + 1) * P],
                                    rhs=mask_t[:],
                                    start=first, stop=False,
                                )
                                first = False
                    # self terms into the 4 column slices
                    for ti in range(ST):
                        t = st * ST + ti
                        nc.tensor.matmul(
                            out=out_ps[:, ti * P:(ti + 1) * P],
                            lhsT=Wr_sb[:], rhs=h_prev[:, t * P:(t + 1) * P],
                            start=first, stop=True,
                        )
                    first = False

                    if layer < 2:
                        for ti in range(ST):
                            t = st * ST + ti
                            nc.scalar.activation(
                                h_new[:, t * P:(t + 1) * P],
                                out_ps[:, ti * P:(ti + 1) * P],
                                mybir.ActivationFunctionType.Relu,
                                bias=bl_sb[:, :1],
                            )
                            m_ps = ps_m.tile([P, HID], F32, tag="mps")
                            nc.tensor.matmul(
                                out=m_ps[:], lhsT=h_new[:, t * P:(t + 1) * P],
                                rhs=Wl_next[:], start=True, stop=True,
                            )
                            m_sb = sb2.tile([P, HID], F16, tag="msb")
                            nc.vector.tensor_copy(m_sb[:], m_ps[:])
                            nc.sync.dma_start(
                                slabs[layer + 1][t * P:(t + 1) * P, :], m_sb[:])
                    else:
                        h3_sb = sb2.tile([HID, ST * P], F16, tag="h3")
                        nc.vector.tensor_scalar(
                            out=h3_sb[:], in0=out_ps[:],
                            scalar1=bl_sb[:, :1], scalar2=None,
                            op0=mybir.AluOpType.add,
                        )
                        for ti in range(ST):
                            t = st * ST + ti
                            h3rm_ps = ps_m.tile([P, HID], F16, tag="h3rm")
                            nc.tensor.transpose(
                                h3rm_ps[:], h3_sb[:, ti * P:(ti + 1) * P],
                                ident16[:])
                            h3rm_sb = sb2.tile([P, HID], F16, tag="h3rmsb")
                            nc.vector.tensor_copy(h3rm_sb[:], h3rm_ps[:])
                            nc.tensor.matmul(
                                out=poolT_ps[:], lhsT=h3rm_sb[:],
                                rhs=gmask_sb[:, t * GPC:(t + 1) * GPC],
                                start=(t == 0), stop=(t == NT - 1),
                            )
                if "nobarrier" not in ABLATE:
                    tc.no_sync_barrier()

            if layer < 2 and "noag" not in ABLATE:
                nc.gpsimd.collective_compute(
                    "AllGather", mybir.AluOpType.bypass, replica_groups=rg,
                    ins=[slabs[layer + 1].opt()], outs=[tbls[layer + 1].opt()],
                )
                tc.no_sync_barrier()

        # ---- head --------------------------------------------------------
        poolT_sb = sb.tile([HID, GPC], F16)
        nc.vector.tensor_copy(poolT_sb[:], poolT_ps[:])
        fin_ps = pool_ps.tile([cfg.num_classes, GPC], F32, tag="fin")
        nc.tensor.matmul(
            out=fin_ps[:], lhsT=Wlin_sb[:], rhs=poolT_sb[:], start=True, stop=True,
        )
        fin_sb = sb.tile([cfg.num_classes, GPC], F32)
        nc.vector.tensor_scalar(
            out=fin_sb[:], in0=fin_ps[:],
            scalar1=blin_sb[:, :1], scalar2=None,
            op0=mybir.AluOpType.add,
        )
        nc.sync.dma_start(out_d[:, :], fin_sb[:])

    nc.compile()
    return nc


# --------------------------------------------------------------------------
# Host-side preprocessing
# --------------------------------------------------------------------------

def preprocess(x, edge_index, batch, cfg_overrides=None):
    num_nodes = x.shape[0]
    in_feat = x.shape[1]
    num_edges = edge_index.shape[1]
    batch = np.asarray(batch, dtype=np.int64)
    src_all = np.asarray(edge_index[0], dtype=np.int64)
    dst_all = np.asarray(edge_index[1], dtype=np.int64)
    n_cores = 8
    num_graphs = int(cfg_overrides.get("num_graphs")) if cfg_overrides and "num_graphs" in cfg_overrides else 512
    gpc = num_graphs // n_cores

    bounds = np.searchsorted(batch, np.arange(n_cores + 1) * gpc)
    nl = bounds[1:] - bounds[:-1]
    blk = ST * P
    n_own = int(-(-int(nl.max()) // blk) * blk)
    assert 2 * n_own <= 32767, "int16 chunk limit"
    chrows = 2 * n_own
    nt = n_own // P
    n_st = nt // ST

    deg = np.bincount(dst_all, minlength=num_nodes)
    w_all = np.zeros(num_edges, np.float32)
    nz = deg[dst_all] > 0
    w_all[nz] = 1.0 / deg[dst_all[nz]]

    owner_d = (batch[dst_all] // gpc).astype(np.int64)
    owner_s = (batch[src_all] // gpc).astype(np.int64)
    src_row = (owner_s * n_own + (src_all - bounds[owner_s])).astype(np.int64)
    chunk = src_row // chrows
    src_rel = (src_row - chunk * chrows).astype(np.int16)
    ld = (dst_all - bounds[owner_d]).astype(np.int64)
    tile_of = ld // P
    st_of = tile_of // ST

    # exact per-(core, st, chunk) counts; program uses max over cores
    gkey = (owner_d * n_st + st_of) * CH + chunk
    ngroups = n_cores * n_st * CH
    gcounts = np.bincount(gkey, minlength=ngroups)
    cnt3 = gcounts.reshape(n_cores, n_st, CH)
    sub_max = -(-cnt3.max(axis=0) // P)          # [n_st, CH] ceil
    sub_max_flat = tuple(int(v) for v in sub_max.reshape(-1))

    cfg = Cfg(
        n_cores=n_cores, num_nodes=num_nodes, num_edges=num_edges,
        in_feat=in_feat, hidden=128, num_graphs=num_graphs,
        num_classes=2, n_own=n_own, sub_max=sub_max_flat, gpc=gpc,
    )
    gcol, subid, call_cols, batch_col0 = _layout(cfg)
    TOTCOL = cfg.tot_sub
    e_proc = TOTCOL * P

    # slot assignment: rank within (core, st, chunk) group
    order = np.argsort(gkey, kind="stable")
    gk_sorted = gkey[order]
    group_start = np.zeros(ngroups, np.int64)
    group_start[1:] = np.cumsum(gcounts)[:-1]
    rank = np.arange(num_edges) - group_start[gk_sorted]
    st_s = (gk_sorted // CH) % n_st
    c_s = gk_sorted % CH
    core_s = gk_sorted // (n_st * CH)
    col = gcol[st_s, c_s] + rank // P
    slot = col * P + rank % P
    row = rank % P

    idx_arr = np.zeros((n_cores, e_proc), np.int16)
    idx_arr[core_s, slot] = src_rel[order]

    # per-slot dst position within the 512-wide supertile + weight, fp16,
    # indexed by mask sub-tile id (subid order), partition = slot % 128
    # (padding slots: dstrel=-1000 -> is_equal never fires, w=0)
    sub_id_edge = subid[st_s, c_s] + rank // P
    dstrel_arr = np.full((n_cores, P, TOTCOL), -1000.0, np.float32)
    w_arr = np.zeros((n_cores, P, TOTCOL), np.float32)
    dpos = (tile_of[order] % ST) * P + (ld[order] - tile_of[order] * P)
    dstrel_arr[core_s, row, sub_id_edge] = dpos.astype(np.float32)
    w_arr[core_s, row, sub_id_edge] = w_all[order]

    def to_i16(a):
        band = a.reshape(e_proc // 16, 16).T
        return np.ascontiguousarray(np.tile(band, (8, 1)))

    gsizes = np.bincount(batch, minlength=num_graphs).astype(np.float32)
    per_core = []
    for c in range(n_cores):
        n0, n1 = int(bounds[c]), int(bounds[c + 1])
        xT = np.zeros((in_feat, n_own), np.float16)
        xT[:, : n1 - n0] = x[n0:n1].T.astype(np.float16)
        # gmask [128, NT*GPC]: node tile t, graph col g -> (batch==g)/graphsize
        gm = np.zeros((n_own, gpc), np.float16)
        loc = np.arange(n1 - n0)
        grel = (batch[n0:n1] - c * gpc).astype(np.int64)
        gs = gsizes[batch[n0:n1]]
        val = np.zeros(n1 - n0, np.float32)
        val[gs > 0] = 1.0 / gs[gs > 0]
        gm[loc, grel] = val.astype(np.float16)
        gmask = np.ascontiguousarray(
            gm.reshape(nt, P, gpc).transpose(1, 0, 2).reshape(P, nt * gpc))

        per_core.append(dict(
            xT=xT,
            eidx=to_i16(idx_arr[c]),
            edstrel=np.ascontiguousarray(dstrel_arr[c]),
            ew=np.ascontiguousarray(w_arr[c]),
            gmask=gmask,
        ))

    return cfg, per_core


def make_in_maps(cfg, per_core, weights):
    wmap = {}
    for k in ("Wl1", "Wr1", "Wl2", "Wr2", "Wl3", "Wr3", "Wlin"):
        wmap[k] = np.ascontiguousarray(weights[k].astype(np.float16))
    for k in ("bl1", "bl2", "bl3", "blin"):
        wmap[k] = np.ascontiguousarray(weights[k].astype(np.float32).reshape(-1, 1))
    in_maps = []
    for c in range(cfg.n_cores):
        m = dict(per_core[c])
        m.update(wmap)
        in_maps.append(m)
    return in_maps


_PROGRAM_CACHE = {}


def kernel(x, edge_index, batch,
           Wl1, bl1, Wr1, Wl2, bl2, Wr2, Wl3, bl3, Wr3, Wlin, blin):
    x = np.asarray(x)
    cfg, per_core = preprocess(np.asarray(x, np.float32),
                               np.asarray(edge_index), np.asarray(batch))
    weights = dict(Wl1=np.asarray(Wl1), bl1=np.asarray(bl1), Wr1=np.asarray(Wr1),
                   Wl2=np.asarray(Wl2), bl2=np.asarray(bl2), Wr2=np.asarray(Wr2),
                   Wl3=np.asarray(Wl3), bl3=np.asarray(bl3), Wr3=np.asarray(Wr3),
                   Wlin=np.asarray(Wlin), blin=np.asarray(blin))
    in_maps = make_in_maps(cfg, per_core, weights)

    key = (cfg.n_own, cfg.sub_max, cfg.in_feat, cfg.num_graphs)
    if key not in _PROGRAM_CACHE:
        _PROGRAM_CACHE[key] = build_program(cfg)
    nc = _PROGRAM_CACHE[key]

    res = bass_utils.run_bass_kernel_spmd(
        nc, in_maps, core_ids=list(range(cfg.n_cores)),
    )
    out = np.empty((cfg.num_graphs, cfg.num_classes), np.float32)
    for c in range(cfg.n_cores):
        out[c * cfg.gpc:(c + 1) * cfg.gpc, :] = res.results[c]["out"].T
    return out


# revision 30
# speedup vs baseline: 1.2915x; 1.2915x over previous
"""Distributed GraphSAGE kernel for Trainium2 (8 NeuronCores, Bass/Tile). v2

Takes FULL inputs (same keys as setup_inputs()), shards by graph id across 8
cores, runs a single SPMD Bass program (3 SAGE layers + global mean pool +
linear head) with inter-layer AllGathers, returns the FULL [512, 2] output.

v2 changes vs v1:
  - one-hot scatter masks (iota==dst)*1/deg are PRECOMPUTED ON HOST and
    streamed from DRAM as matmul rhs operands (v1 built them per-subtile on
    DVE: ~3.8ms of vector-engine time, the top bottleneck)
  - aggregation runs on 512-wide supertiles (4 node tiles per PSUM tile) with
    EXACT per-(supertile,chunk) edge sub-tile counts (max over the 8 cores so
    the SPMD program is shared); v1 used a global worst-case te_c budget
    (padding 153k slots/core/layer -> 111k)
  - h^T slabs live in SBUF (no DRAM round trip between layers)
  - graph-pool masks precomputed on host as well
  - per-batch (not per-tile) scheduler barriers

Algorithm per core (nodes sharded by graph; batch sorted so each core owns a
contiguous node range; edges assigned to the core owning their dst):
  - table TBL_l holds m_l = h_{l-1} @ Wl_l for ALL nodes (fp16, allgathered);
    TBL split into 4 row-chunks of 2*n_own rows for int16 dma_gather indices
  - per gather batch (NBS supertiles): 4 dma_gather calls (one per chunk)
    with exact slot counts; per supertile: mask-matmuls accumulate
    (mean_agg @ Wl_l)^T into a [128, 512] PSUM tile, self terms
    Wr_l^T @ h^T accumulate into the 4 column slices
  - relu+bias on ACT -> h_l^T slab (SBUF); m_{l+1} matmul per tile -> slab ->
    AllGather
  - layer 3: bias on DVE, transpose tiles via TensorE, pool with precomputed
    per-graph masks, then Wlin matmul + bias
"""
import sys
import os

sys.path.insert(0, "/opt/trn_rl_repo")

import numpy as np
from contextlib import ExitStack
from dataclasses import dataclass

from concourse import bass, mybir, tile, bacc
from concourse import bass_utils
from concourse.masks import make_identity

P = 128
CH = 4              # table row chunks (int16 index limit)
ST = 4              # node tiles per supertile (512-wide PSUM)
BATCH_ST = 3        # supertiles per gather batch (last batch may be smaller)
F16 = mybir.dt.float16
F32 = mybir.dt.float32
I16 = mybir.dt.int16

ABLATE = frozenset()


@dataclass(frozen=True)
class Cfg:
    n_cores: int
    num_nodes: int
    num_edges: int
    in_feat: int
    hidden: int
    num_graphs: int
    num_classes: int
    n_own: int             # padded nodes per core (multiple of ST*128)
    sub_max: tuple         # flat tuple, sub_max[st*CH+c] subtiles per (st,chunk)
    gpc: int               # graphs per core

    @property
    def nt(self):
        return self.n_own // P

    @property
    def n_st(self):
        return self.nt // ST

    @property
    def batches(self):
        """List of (st0, n_st_in_batch)."""
        out = []
        st = 0
        while st < self.n_st:
            n = min(BATCH_ST, self.n_st - st)
            out.append((st, n))
            st += n
        return out

    @property
    def sub_arr(self):
        return np.asarray(self.sub_max, np.int64).reshape(self.n_st, CH)

    @property
    def tot_sub(self):
        return int(self.sub_arr.sum())


def _layout(cfg: Cfg):
    """Static layout tables shared by host preprocessing and program build.

    Returns:
      gcol   [n_st, CH]: first g_t column (within the layer-global column
             space, order (batch, chunk, st, j)) of each (st, chunk) group
      subid  [n_st, CH]: first mask sub-tile id (order (batch, st, chunk, j))
      call_cols [n_batch, CH]: columns per dma_gather call
      batch_col0 [n_batch]: first global column of each batch
    """
    sub = cfg.sub_arr
    batches = cfg.batches
    n_b = len(batches)
    gcol = np.zeros((cfg.n_st, CH), np.int64)
    subid = np.zeros((cfg.n_st, CH), np.int64)
    call_cols = np.zeros((n_b, CH), np.int64)
    batch_col0 = np.zeros(n_b, np.int64)
    col = 0
    for b, (st0, nb) in enumerate(batches):
        batch_col0[b] = col
        for c in range(CH):
            for s in range(nb):
                st = st0 + s
                gcol[st, c] = col
                col += sub[st, c]
            call_cols[b, c] = int(sub[st0:st0 + nb, c].sum())
    sid = 0
    for st in range(cfg.n_st):
        for c in range(CH):
            subid[st, c] = sid
            sid += sub[st, c]
    return gcol, subid, call_cols, batch_col0


def build_program(cfg: Cfg):
    nc = bacc.Bacc(
        "TRN2",
        target_bir_lowering=False,
        debug=False,
        num_devices=cfg.n_cores,
        num_swdge_queues=1,
    )

    NT = cfg.nt
    HID = cfg.hidden
    INF = cfg.in_feat
    GPC = cfg.gpc
    NC = cfg.n_cores
    CHROWS = 2 * cfg.n_own
    sub = cfg.sub_arr
    batches = cfg.batches
    n_b = len(batches)
    gcol, subid, call_cols, batch_col0 = _layout(cfg)
    TOTCOL = int(sub.sum())
    max_batch_cols = int(max(
        (batch_col0[b + 1] if b + 1 < n_b else TOTCOL) - batch_col0[b]
        for b in range(n_b)
    ))
    max_stc_sub = int(sub.max())

    # ---- I/O -------------------------------------------------------------
    xT_d = nc.dram_tensor("xT", [INF, cfg.n_own], F16, kind="ExternalInput")
    idx_d = nc.dram_tensor("eidx", [P, TOTCOL * 8], I16, kind="ExternalInput")
    # dstrel in mask-subtile (subid) order; per-slot weight in g_t (gcol) order
    dstrel_d = nc.dram_tensor("edstrel", [P, TOTCOL], F16, kind="ExternalInput")
    wg_d = nc.dram_tensor("ewg", [P, TOTCOL], F16, kind="ExternalInput")
    gmask_d = nc.dram_tensor("gmask", [P, NT * GPC], F16, kind="ExternalInput")
    Wl1_d = nc.dram_tensor("Wl1", [INF, HID], F16, kind="ExternalInput")
    Wr1_d = nc.dram_tensor("Wr1", [INF, HID], F16, kind="ExternalInput")
    Wl2_d = nc.dram_tensor("Wl2", [HID, HID], F16, kind="ExternalInput")
    Wr2_d = nc.dram_tensor("Wr2", [HID, HID], F16, kind="ExternalInput")
    Wl3_d = nc.dram_tensor("Wl3", [HID, HID], F16, kind="ExternalInput")
    Wr3_d = nc.dram_tensor("Wr3", [HID, HID], F16, kind="ExternalInput")
    Wlin_d = nc.dram_tensor("Wlin", [HID, cfg.num_classes], F16, kind="ExternalInput")
    bl1_d = nc.dram_tensor("bl1", [HID, 1], F32, kind="ExternalInput")
    bl2_d = nc.dram_tensor("bl2", [HID, 1], F32, kind="ExternalInput")
    bl3_d = nc.dram_tensor("bl3", [HID, 1], F32, kind="ExternalInput")
    blin_d = nc.dram_tensor("blin", [cfg.num_classes, 1], F32, kind="ExternalInput")
    out_d = nc.dram_tensor("out", [cfg.num_classes, GPC], F32, kind="ExternalOutput")

    rg = [list(range(NC))]

    with tile.TileContext(nc) as tc, ExitStack() as ctx:
        sb = ctx.enter_context(tc.tile_pool(name="sb", bufs=1))
        hpool = ctx.enter_context(tc.tile_pool(name="hp", bufs=2))
        sb2 = ctx.enter_context(tc.tile_pool(name="sb2", bufs=4))
        mpool = ctx.enter_context(tc.tile_pool(name="mp", bufs=2))
        gbuf = ctx.enter_context(tc.tile_pool(name="gbuf", bufs=2))
        ps = ctx.enter_context(tc.tile_pool(name="ps", bufs=2, space="PSUM"))
        ps_m = ctx.enter_context(tc.tile_pool(name="psm", bufs=2, space="PSUM"))
        pool_ps = ctx.enter_context(tc.tile_pool(name="pps", bufs=1, space="PSUM"))
        dram = ctx.enter_context(tc.tile_pool(name="dram", bufs=1, space="DRAM"))

        # ---- static SBUF state ------------------------------------------
        ident16 = sb.tile([P, P], F16)
        make_identity(nc, ident16[:])

        # iota over the 512 supertile columns, fp16 (exact for 0..511)
        iota_i = sb.tile([P, ST * P], mybir.dt.int32)
        nc.gpsimd.iota(iota_i[:], pattern=[[1, ST * P]], base=0,
                       channel_multiplier=0)
        iota_f = sb.tile([P, ST * P], F16)
        nc.vector.tensor_copy(iota_f[:], iota_i[:])

        idx_sb = sb.tile([P, TOTCOL * 8], I16)
        nc.sync.dma_start(idx_sb[:], idx_d[:, :])
        dstrel_sb = sb.tile([P, TOTCOL], F16)
        nc.sync.dma_start(dstrel_sb[:], dstrel_d[:, :])
        wg_sb = sb.tile([P, TOTCOL], F16)
        nc.sync.dma_start(wg_sb[:], wg_d[:, :])
        gmask_sb = sb.tile([P, NT * GPC], F16)
        nc.sync.dma_start(gmask_sb[:], gmask_d[:, :])

        def load_w(d, p_, f_, nm):
            t = sb.tile([p_, f_], F16, name=nm, tag=nm)
            nc.sync.dma_start(t[:], d[:, :])
            return t

        Wl1_sb = load_w(Wl1_d, INF, HID, "wl1s")
        Wr1_sb = load_w(Wr1_d, INF, HID, "wr1s")
        Wl2_sb = load_w(Wl2_d, HID, HID, "wl2s")
        Wr2_sb = load_w(Wr2_d, HID, HID, "wr2s")
        Wl3_sb = load_w(Wl3_d, HID, HID, "wl3s")
        Wr3_sb = load_w(Wr3_d, HID, HID, "wr3s")
        Wlin_sb = load_w(Wlin_d, HID, cfg.num_classes, "wlins")
        bl1_sb = sb.tile([HID, 1], F32)
        nc.sync.dma_start(bl1_sb[:], bl1_d[:, :])
        bl2_sb = sb.tile([HID, 1], F32)
        nc.sync.dma_start(bl2_sb[:], bl2_d[:, :])
        bl3_sb = sb.tile([HID, 1], F32)
        nc.sync.dma_start(bl3_sb[:], bl3_d[:, :])
        blin_sb = sb.tile([cfg.num_classes, 1], F32)
        nc.sync.dma_start(blin_sb[:], blin_d[:, :])

        # xT and the two h^T slabs share one 2-slot tag: ht1 reuses xT's slot
        # once layer 1 (the last xT reader) is done
        xT_sb = hpool.tile([INF, cfg.n_own], F16, tag="hx", name="xT")
        nc.sync.dma_start(xT_sb[:], xT_d[:, :])
        tc.no_sync_barrier()

        # ---- internal DRAM ----------------------------------------------
        slabs = [dram.tile([cfg.n_own, HID], F16, tag=f"slab{l}", name=f"slab{l}")
                 for l in range(3)]
        tbls = [dram.tile([NC * cfg.n_own, HID], F16, tag=f"tbl{l}",
                          name=f"tbl{l}", addr_space="Shared")
                for l in range(3)]

        # h^T slabs stay in SBUF between layers (allocated lazily per layer)
        hts = [None, None]

        # ---- P0: m1 = x @ Wl1 (row-major slab) --------------------------
        for t in range(NT):
            m_ps = ps_m.tile([P, HID], F32, tag="mps")
            nc.tensor.matmul(
                out=m_ps[:], lhsT=xT_sb[:, t * P:(t + 1) * P], rhs=Wl1_sb[:],
                start=True, stop=True,
            )
            m_sb = sb2.tile([P, HID], F16, tag="msb")
            nc.vector.tensor_copy(m_sb[:], m_ps[:])
            nc.sync.dma_start(slabs[0][t * P:(t + 1) * P, :], m_sb[:])

        if "noag" not in ABLATE:
            nc.gpsimd.collective_compute(
                "AllGather", mybir.AluOpType.bypass, replica_groups=rg,
                ins=[slabs[0].opt()], outs=[tbls[0].opt()],
            )
        tc.no_sync_barrier()

        # ---- layers ------------------------------------------------------
        for layer in range(3):
            tbl = tbls[layer]
            Wr_sb = (Wr1_sb, Wr2_sb, Wr3_sb)[layer]
            bl_sb = (bl1_sb, bl2_sb, bl3_sb)[layer]
            Wl_next = (Wl2_sb, Wl3_sb, None)[layer]
            if layer < 2:
                hts[layer] = hpool.tile([HID, cfg.n_own], F16, tag="hx",
                                        name=f"ht{layer}")
            h_prev = xT_sb if layer == 0 else hts[layer - 1]
            h_new = hts[layer] if layer < 2 else None

            if layer == 2:
                poolT_ps = pool_ps.tile([HID, GPC], F32, tag="pool")

            for b, (st0, nb_st) in enumerate(batches):
                bc0 = int(batch_col0[b])
                g_t = gbuf.tile([P, max_batch_cols * P], F16, tag="g")
                rel = 0
                for c in range(CH):
                    ncols = int(call_cols[b, c])
                    if ncols == 0 or "nogather" in ABLATE:
                        rel += ncols
                        continue
                    col0 = bc0 + rel
                    nidx = ncols * P
                    nc.gpsimd.dma_gather(
                        out_ap=g_t[:, rel * P:(rel + ncols) * P].rearrange(
                            "p (t e) -> p t e", e=HID),
                        in_ap=tbl[c * CHROWS:(c + 1) * CHROWS, :],
                        idxs_ap=idx_sb[:, col0 * 8:(col0 + ncols) * 8],
                        num_idxs=nidx,
                        num_idxs_reg=nidx,
                        elem_size=HID,
                        single_packet=False,
                    )
                    # fold the per-edge weight into the gathered rows
                    # (in place, one batched op per gather call)
                    nc.vector.tensor_tensor(
                        out=g_t[:, rel * P:(rel + ncols) * P].rearrange(
                            "p (n s) -> p n s", s=P),
                        in0=g_t[:, rel * P:(rel + ncols) * P].rearrange(
                            "p (n s) -> p n s", s=P),
                        in1=wg_sb[:, col0:col0 + ncols].rearrange(
                            "p (n o) -> p n o", o=1).broadcast_to(
                            [P, ncols, P]),
                        op=mybir.AluOpType.mult,
                    )
                    rel += ncols

                for s in range(nb_st):
                    st = st0 + s
                    out_ps = ps.tile([HID, ST * P], F32, tag="outT")
                    first = True
                    if "nomaskmm" not in ABLATE:
                        for c in range(CH):
                            nsub = int(sub[st, c])
                            if nsub == 0:
                                continue
                            # one batched 0/1 mask build for all sub-tiles of
                            # this (supertile, chunk): (iota == dstrel)
                            k0 = int(subid[st, c])
                            mask_t = mpool.tile([P, max_stc_sub * ST * P], F16,
                                                tag="mask")
                            nc.vector.tensor_tensor(
                                out=mask_t[:, :nsub * ST * P].rearrange(
                                    "p (n s) -> p n s", s=ST * P),
                                in0=iota_f[:].rearrange(
                                    "p (o s) -> p o s", o=1).broadcast_to(
                                    [P, nsub, ST * P]),
                                in1=dstrel_sb[:, k0:k0 + nsub].rearrange(
                                    "p (n o) -> p n o", o=1).broadcast_to(
                                    [P, nsub, ST * P]),
                                op=mybir.AluOpType.is_equal,
                            )
                            for j in range(nsub):
                                k_rel = int(gcol[st, c]) - bc0 + j
                                nc.tensor.matmul(
                                    out=out_ps[:],
                                    lhsT=g_t[:, k_rel * P:(k_rel + 1) * P],
                                    rhs=mask_t[:, j * ST * P:(j + 1) * ST * P],
                                    start=first, stop=False,
                                )
                                first = False
                    # self terms into the 4 column slices
                    for ti in range(ST):
                        t = st * ST + ti
                        nc.tensor.matmul(
                            out=out_ps[:, ti * P:(ti + 1) * P],
                            lhsT=Wr_sb[:], rhs=h_prev[:, t * P:(t + 1) * P],
                            start=first, stop=True,
                        )
                    first = False

                    if layer < 2:
                        for ti in range(ST):
                            t = st * ST + ti
                            nc.scalar.activation(
                                h_new[:, t * P:(t + 1) * P],
                                out_ps[:, ti * P:(ti + 1) * P],
                                mybir.ActivationFunctionType.Relu,
                                bias=bl_sb[:, :1],
                            )
                            m_ps = ps_m.tile([P, HID], F32, tag="mps")
                            nc.tensor.matmul(
                                out=m_ps[:], lhsT=h_new[:, t * P:(t + 1) * P],
                                rhs=Wl_next[:], start=True, stop=True,
                            )
                            m_sb = sb2.tile([P, HID], F16, tag="msb")
                            nc.vector.tensor_copy(m_sb[:], m_ps[:])
                            nc.sync.dma_start(
                                slabs[layer + 1][t * P:(t + 1) * P, :], m_sb[:])
                    else:
                        h3_sb = sb2.tile([HID, ST * P], F16, tag="h3")
                        nc.vector.tensor_scalar(
                            out=h3_sb[:], in0=out_ps[:],
                            scalar1=bl_sb[:, :1], scalar2=None,
                            op0=mybir.AluOpType.add,
                        )
                        for ti in range(ST):
                            t = st * ST + ti
                            h3rm_ps = ps_m.tile([P, HID], F16, tag="h3rm")
                            nc.tensor.transpose(
                                h3rm_ps[:], h3_sb[:, ti * P:(ti + 1) * P],
                                ident16[:])
                            h3rm_sb = sb2.tile([P, HID], F16, tag="h3rmsb")
                            nc.vector.tensor_copy(h3rm_sb[:], h3rm_ps[:])
                            nc.tensor.matmul(
                                out=poolT_ps[:], lhsT=h3rm_sb[:],
                                rhs=gmask_sb[:, t * GPC:(t + 1) * GPC],
                                start=(t == 0), stop=(t == NT - 1),
                            )
                if "nobarrier" not in ABLATE:
                    tc.no_sync_barrier()

            if layer < 2 and "noag" not in ABLATE:
                nc.gpsimd.collective_compute(
                    "AllGather", mybir.AluOpType.bypass, replica_groups=rg,
                    ins=[slabs[layer + 1].opt()], outs=[tbls[layer + 1].opt()],
                )
                tc.no_sync_barrier()

        # ---- head --------------------------------------------------------
        poolT_sb = sb.tile([HID, GPC], F16)
        nc.vector.tensor_copy(poolT_sb[:], poolT_ps[:])
        fin_ps = pool_ps.tile([cfg.num_classes, GPC], F32, tag="fin")
        nc.tensor.matmul(
            out=fin_ps[:], lhsT=Wlin_sb[:], rhs=poolT_sb[:], start=True, stop=True,
        )
        fin_sb = sb.tile([cfg.num_classes, GPC], F32)
        nc.vector.tensor_scalar(
            out=fin_sb[:], in0=fin_ps[:],
            scalar1=blin_sb[:, :1], scalar2=None,
            op0=mybir.AluOpType.add,
        )
        nc.sync.dma_start(out_d[:, :], fin_sb[:])

    nc.compile()
    return nc


# --------------------------------------------------------------------------
# Host-side preprocessing
# --------------------------------------------------------------------------

def preprocess(x, edge_index, batch, cfg_overrides=None):
    num_nodes = x.shape[0]
    in_feat = x.shape[1]
    num_edges = edge_index.shape[1]
    batch = np.asarray(batch, dtype=np.int64)
    src_all = np.asarray(edge_index[0], dtype=np.int64)
    dst_all = np.asarray(edge_index[1], dtype=np.int64)
    n_cores = 8
    num_graphs = int(cfg_overrides.get("num_graphs")) if cfg_overrides and "num_graphs" in cfg_overrides else 512
    gpc = num_graphs // n_cores

    bounds = np.searchsorted(batch, np.arange(n_cores + 1) * gpc)
    nl = bounds[1:] - bounds[:-1]
    blk = ST * P
    n_own = int(-(-int(nl.max()) // blk) * blk)
    assert 2 * n_own <= 32767, "int16 chunk limit"
    chrows = 2 * n_own
    nt = n_own // P
    n_st = nt // ST

    deg = np.bincount(dst_all, minlength=num_nodes)
    w_all = np.zeros(num_edges, np.float32)
    nz = deg[dst_all] > 0
    w_all[nz] = 1.0 / deg[dst_all[nz]]

    owner_d = (batch[dst_all] // gpc).astype(np.int64)
    owner_s = (batch[src_all] // gpc).astype(np.int64)
    src_row = (owner_s * n_own + (src_all - bounds[owner_s])).astype(np.int64)
    chunk = src_row // chrows
    src_rel = (src_row - chunk * chrows).astype(np.int16)
    ld = (dst_all - bounds[owner_d]).astype(np.int64)
    tile_of = ld // P
    st_of = tile_of // ST

    # exact per-(core, st, chunk) counts; program uses max over cores
    gkey = (owner_d * n_st + st_of) * CH + chunk
    ngroups = n_cores * n_st * CH
    gcounts = np.bincount(gkey, minlength=ngroups)
    cnt3 = gcounts.reshape(n_cores, n_st, CH)
    sub_max = -(-cnt3.max(axis=0) // P)          # [n_st, CH] ceil
    sub_max_flat = tuple(int(v) for v in sub_max.reshape(-1))

    cfg = Cfg(
        n_cores=n_cores, num_nodes=num_nodes, num_edges=num_edges,
        in_feat=in_feat, hidden=128, num_graphs=num_graphs,
        num_classes=2, n_own=n_own, sub_max=sub_max_flat, gpc=gpc,
    )
    gcol, subid, call_cols, batch_col0 = _layout(cfg)
    TOTCOL = cfg.tot_sub
    e_proc = TOTCOL * P

    # slot assignment: rank within (core, st, chunk) group
    order = np.argsort(gkey, kind="stable")
    gk_sorted = gkey[order]
    group_start = np.zeros(ngroups, np.int64)
    group_start[1:] = np.cumsum(gcounts)[:-1]
    rank = np.arange(num_edges) - group_start[gk_sorted]
    st_s = (gk_sorted // CH) % n_st
    c_s = gk_sorted % CH
    core_s = gk_sorted // (n_st * CH)
    col = gcol[st_s, c_s] + rank // P
    slot = col * P + rank % P
    row = rank % P

    idx_arr = np.zeros((n_cores, e_proc), np.int16)
    idx_arr[core_s, slot] = src_rel[order]

    # dstrel (subid order): dst position within the 512-wide supertile;
    # padding slots -1000 so is_equal never fires.
    # wg (gcol order): per-slot weight folded into the gathered rows.
    sub_id_edge = subid[st_s, c_s] + rank // P
    dstrel_arr = np.full((n_cores, P, TOTCOL), -1000.0, np.float16)
    w_arr = np.zeros((n_cores, P, TOTCOL), np.float16)
    dpos = (tile_of[order] % ST) * P + (ld[order] - tile_of[order] * P)
    dstrel_arr[core_s, row, sub_id_edge] = dpos.astype(np.float16)
    w_arr[core_s, row, col] = w_all[order].astype(np.float16)

    def to_i16(a):
        band = a.reshape(e_proc // 16, 16).T
        return np.ascontiguousarray(np.tile(band, (8, 1)))

    gsizes = np.bincount(batch, minlength=num_graphs).astype(np.float32)
    per_core = []
    for c in range(n_cores):
        n0, n1 = int(bounds[c]), int(bounds[c + 1])
        xT = np.zeros((in_feat, n_own), np.float16)
        xT[:, : n1 - n0] = x[n0:n1].T.astype(np.float16)
        # gmask [128, NT*GPC]: node tile t, graph col g -> (batch==g)/graphsize
        gm = np.zeros((n_own, gpc), np.float16)
        loc = np.arange(n1 - n0)
        grel = (batch[n0:n1] - c * gpc).astype(np.int64)
        gs = gsizes[batch[n0:n1]]
        val = np.zeros(n1 - n0, np.float32)
        val[gs > 0] = 1.0 / gs[gs > 0]
        gm[loc, grel] = val.astype(np.float16)
        gmask = np.ascontiguousarray(
            gm.reshape(nt, P, gpc).transpose(1, 0, 2).reshape(P, nt * gpc))

        per_core.append(dict(
            xT=xT,
            eidx=to_i16(idx_arr[c]),
            edstrel=np.ascontiguousarray(dstrel_arr[c]),
            ewg=np.ascontiguousarray(w_arr[c]),
            gmask=gmask,
        ))

    return cfg, per_core


def make_in_maps(cfg, per_core, weights):
    wmap = {}
    for k in ("Wl1", "Wr1", "Wl2", "Wr2", "Wl3", "Wr3", "Wlin"):
        wmap[k] = np.ascontiguousarray(weights[k].astype(np.float16))
    for k in ("bl1", "bl2", "bl3", "blin"):
        wmap[k] = np.ascontiguousarray(weights[k].astype(np.float32).reshape(-1, 1))
    in_maps = []
    for c in range(cfg.n_cores):
        m = dict(per_core[c])
        m.update(wmap)
        in_maps.append(m)
    return in_maps


_PROGRAM_CACHE = {}


def kernel(x, edge_index, batch,
           Wl1, bl1, Wr1, Wl2, bl2, Wr2, Wl3, bl3, Wr3, Wlin, blin):
    x = np.asarray(x)
    cfg, per_core = preprocess(np.asarray(x, np.float32),
                               np.asarray(edge_index), np.asarray(batch))
    weights = dict(Wl1=np.asarray(Wl1), bl1=np.asarray(bl1), Wr1=np.asarray(Wr1),
                   Wl2=np.asarray(Wl2), bl2=np.asarray(bl2), Wr2=np.asarray(Wr2),
                   Wl3=np.asarray(Wl3), bl3=np.asarray(bl3), Wr3=np.asarray(Wr3),
                   Wlin=np.asarray(Wlin), blin=np.asarray(blin))
    in_maps = make_in_maps(cfg, per_core, weights)

    key = (cfg.n_own, cfg.sub_max, cfg.in_feat, cfg.num_graphs)
    if key not in _PROGRAM_CACHE:
        _PROGRAM_CACHE[key] = build_program(cfg)
    nc = _PROGRAM_CACHE[key]

    res = bass_utils.run_bass_kernel_spmd(
        nc, in_maps, core_ids=list(range(cfg.n_cores)),
    )
    out = np.empty((cfg.num_graphs, cfg.num_classes), np.float32)
    for c in range(cfg.n_cores):
        out[c * cfg.gpc:(c + 1) * cfg.gpc, :] = res.results[c]["out"].T
    return out


# revision 33
# speedup vs baseline: 1.2962x; 1.0036x over previous
"""Distributed GraphSAGE kernel for Trainium2 (8 NeuronCores, Bass/Tile). v2

Takes FULL inputs (same keys as setup_inputs()), shards by graph id across 8
cores, runs a single SPMD Bass program (3 SAGE layers + global mean pool +
linear head) with inter-layer AllGathers, returns the FULL [512, 2] output.

v2 changes vs v1:
  - one-hot scatter masks (iota==dst)*1/deg are PRECOMPUTED ON HOST and
    streamed from DRAM as matmul rhs operands (v1 built them per-subtile on
    DVE: ~3.8ms of vector-engine time, the top bottleneck)
  - aggregation runs on 512-wide supertiles (4 node tiles per PSUM tile) with
    EXACT per-(supertile,chunk) edge sub-tile counts (max over the 8 cores so
    the SPMD program is shared); v1 used a global worst-case te_c budget
    (padding 153k slots/core/layer -> 111k)
  - h^T slabs live in SBUF (no DRAM round trip between layers)
  - graph-pool masks precomputed on host as well
  - per-batch (not per-tile) scheduler barriers

Algorithm per core (nodes sharded by graph; batch sorted so each core owns a
contiguous node range; edges assigned to the core owning their dst):
  - table TBL_l holds m_l = h_{l-1} @ Wl_l for ALL nodes (fp16, allgathered);
    TBL split into 4 row-chunks of 2*n_own rows for int16 dma_gather indices
  - per gather batch (NBS supertiles): 4 dma_gather calls (one per chunk)
    with exact slot counts; per supertile: mask-matmuls accumulate
    (mean_agg @ Wl_l)^T into a [128, 512] PSUM tile, self terms
    Wr_l^T @ h^T accumulate into the 4 column slices
  - relu+bias on ACT -> h_l^T slab (SBUF); m_{l+1} matmul per tile -> slab ->
    AllGather
  - layer 3: bias on DVE, transpose tiles via TensorE, pool with precomputed
    per-graph masks, then Wlin matmul + bias
"""
import sys
import os

sys.path.insert(0, "/opt/trn_rl_repo")

import numpy as np
from contextlib import ExitStack
from dataclasses import dataclass

from concourse import bass, mybir, tile, bacc
from concourse import bass_utils
from concourse.masks import make_identity

P = 128
CH = 4              # table row chunks (int16 index limit)
ST = 4              # node tiles per supertile (512-wide PSUM)
BATCH_ST = 3        # supertiles per gather batch (last batch may be smaller)
F16 = mybir.dt.float16
F32 = mybir.dt.float32
I16 = mybir.dt.int16

ABLATE = frozenset()


@dataclass(frozen=True)
class Cfg:
    n_cores: int
    num_nodes: int
    num_edges: int
    in_feat: int
    hidden: int
    num_graphs: int
    num_classes: int
    n_own: int             # padded nodes per core (multiple of ST*128)
    sub_max: tuple         # flat tuple, sub_max[st*CH+c] subtiles per (st,chunk)
    gpc: int               # graphs per core

    @property
    def nt(self):
        return self.n_own // P

    @property
    def n_st(self):
        return self.nt // ST

    @property
    def batches(self):
        """List of (st0, n_st_in_batch)."""
        out = []
        st = 0
        while st < self.n_st:
            n = min(BATCH_ST, self.n_st - st)
            out.append((st, n))
            st += n
        return out

    @property
    def sub_arr(self):
        return np.asarray(self.sub_max, np.int64).reshape(self.n_st, CH)

    @property
    def tot_sub(self):
        return int(self.sub_arr.sum())


def _layout(cfg: Cfg):
    """Static layout tables shared by host preprocessing and program build.

    Returns:
      gcol   [n_st, CH]: first g_t column (within the layer-global column
             space, order (batch, chunk, st, j)) of each (st, chunk) group
      subid  [n_st, CH]: first mask sub-tile id (order (batch, st, chunk, j))
      call_cols [n_batch, CH]: columns per dma_gather call
      batch_col0 [n_batch]: first global column of each batch
    """
    sub = cfg.sub_arr
    batches = cfg.batches
    n_b = len(batches)
    gcol = np.zeros((cfg.n_st, CH), np.int64)
    subid = np.zeros((cfg.n_st, CH), np.int64)
    call_cols = np.zeros((n_b, CH), np.int64)
    batch_col0 = np.zeros(n_b, np.int64)
    col = 0
    for b, (st0, nb) in enumerate(batches):
        batch_col0[b] = col
        for c in range(CH):
            for s in range(nb):
                st = st0 + s
                gcol[st, c] = col
                col += sub[st, c]
            call_cols[b, c] = int(sub[st0:st0 + nb, c].sum())
    sid = 0
    for st in range(cfg.n_st):
        for c in range(CH):
            subid[st, c] = sid
            sid += sub[st, c]
    return gcol, subid, call_cols, batch_col0


def build_program(cfg: Cfg):
    nc = bacc.Bacc(
        "TRN2",
        target_bir_lowering=False,
        debug=False,
        num_devices=cfg.n_cores,
        num_swdge_queues=1,
    )

    NT = cfg.nt
    HID = cfg.hidden
    INF = cfg.in_feat
    GPC = cfg.gpc
    NC = cfg.n_cores
    CHROWS = 2 * cfg.n_own
    sub = cfg.sub_arr
    batches = cfg.batches
    n_b = len(batches)
    gcol, subid, call_cols, batch_col0 = _layout(cfg)
    TOTCOL = int(sub.sum())
    max_batch_cols = int(max(
        (batch_col0[b + 1] if b + 1 < n_b else TOTCOL) - batch_col0[b]
        for b in range(n_b)
    ))
    max_stc_sub = int(sub.max())

    # ---- I/O -------------------------------------------------------------
    xT_d = nc.dram_tensor("xT", [INF, cfg.n_own], F16, kind="ExternalInput")
    idx_d = nc.dram_tensor("eidx", [P, TOTCOL * 8], I16, kind="ExternalInput")
    # dstrel in mask-subtile (subid) order; per-slot weight in g_t (gcol) order
    dstrel_d = nc.dram_tensor("edstrel", [P, TOTCOL], F16, kind="ExternalInput")
    wg_d = nc.dram_tensor("ewg", [P, TOTCOL], F32, kind="ExternalInput")
    gmask_d = nc.dram_tensor("gmask", [P, NT * GPC], F16, kind="ExternalInput")
    Wl1_d = nc.dram_tensor("Wl1", [INF, HID], F16, kind="ExternalInput")
    Wr1_d = nc.dram_tensor("Wr1", [INF, HID], F16, kind="ExternalInput")
    Wl2_d = nc.dram_tensor("Wl2", [HID, HID], F16, kind="ExternalInput")
    Wr2_d = nc.dram_tensor("Wr2", [HID, HID], F16, kind="ExternalInput")
    Wl3_d = nc.dram_tensor("Wl3", [HID, HID], F16, kind="ExternalInput")
    Wr3_d = nc.dram_tensor("Wr3", [HID, HID], F16, kind="ExternalInput")
    Wlin_d = nc.dram_tensor("Wlin", [HID, cfg.num_classes], F16, kind="ExternalInput")
    bl1_d = nc.dram_tensor("bl1", [HID, 1], F32, kind="ExternalInput")
    bl2_d = nc.dram_tensor("bl2", [HID, 1], F32, kind="ExternalInput")
    bl3_d = nc.dram_tensor("bl3", [HID, 1], F32, kind="ExternalInput")
    blin_d = nc.dram_tensor("blin", [cfg.num_classes, 1], F32, kind="ExternalInput")
    out_d = nc.dram_tensor("out", [cfg.num_classes, GPC], F32, kind="ExternalOutput")

    rg = [list(range(NC))]

    with tile.TileContext(nc) as tc, ExitStack() as ctx:
        sb = ctx.enter_context(tc.tile_pool(name="sb", bufs=1))
        hpool = ctx.enter_context(tc.tile_pool(name="hp", bufs=2))
        sb2 = ctx.enter_context(tc.tile_pool(name="sb2", bufs=4))
        mpool = ctx.enter_context(tc.tile_pool(name="mp", bufs=2))
        gbuf = ctx.enter_context(tc.tile_pool(name="gbuf", bufs=2))
        ps = ctx.enter_context(tc.tile_pool(name="ps", bufs=2, space="PSUM"))
        ps_m = ctx.enter_context(tc.tile_pool(name="psm", bufs=2, space="PSUM"))
        pool_ps = ctx.enter_context(tc.tile_pool(name="pps", bufs=1, space="PSUM"))
        dram = ctx.enter_context(tc.tile_pool(name="dram", bufs=1, space="DRAM"))

        # ---- static SBUF state ------------------------------------------
        ident16 = sb.tile([P, P], F16)
        make_identity(nc, ident16[:])

        # iota over the 512 supertile columns, fp16 (exact for 0..511)
        iota_i = sb.tile([P, ST * P], mybir.dt.int32)
        nc.gpsimd.iota(iota_i[:], pattern=[[1, ST * P]], base=0,
                       channel_multiplier=0)
        iota_f = sb.tile([P, ST * P], F16)
        nc.vector.tensor_copy(iota_f[:], iota_i[:])

        idx_sb = sb.tile([P, TOTCOL * 8], I16)
        nc.sync.dma_start(idx_sb[:], idx_d[:, :])
        dstrel_sb = sb.tile([P, TOTCOL], F16)
        nc.sync.dma_start(dstrel_sb[:], dstrel_d[:, :])
        wg_sb = sb.tile([P, TOTCOL], F32)
        nc.sync.dma_start(wg_sb[:], wg_d[:, :])
        gmask_sb = sb.tile([P, NT * GPC], F16)
        nc.sync.dma_start(gmask_sb[:], gmask_d[:, :])

        def load_w(d, p_, f_, nm):
            t = sb.tile([p_, f_], F16, name=nm, tag=nm)
            nc.sync.dma_start(t[:], d[:, :])
            return t

        Wl1_sb = load_w(Wl1_d, INF, HID, "wl1s")
        Wr1_sb = load_w(Wr1_d, INF, HID, "wr1s")
        Wl2_sb = load_w(Wl2_d, HID, HID, "wl2s")
        Wr2_sb = load_w(Wr2_d, HID, HID, "wr2s")
        Wl3_sb = load_w(Wl3_d, HID, HID, "wl3s")
        Wr3_sb = load_w(Wr3_d, HID, HID, "wr3s")
        Wlin_sb = load_w(Wlin_d, HID, cfg.num_classes, "wlins")
        bl1_sb = sb.tile([HID, 1], F32)
        nc.sync.dma_start(bl1_sb[:], bl1_d[:, :])
        bl2_sb = sb.tile([HID, 1], F32)
        nc.sync.dma_start(bl2_sb[:], bl2_d[:, :])
        bl3_sb = sb.tile([HID, 1], F32)
        nc.sync.dma_start(bl3_sb[:], bl3_d[:, :])
        blin_sb = sb.tile([cfg.num_classes, 1], F32)
        nc.sync.dma_start(blin_sb[:], blin_d[:, :])

        # xT and the two h^T slabs share one 2-slot tag: ht1 reuses xT's slot
        # once layer 1 (the last xT reader) is done
        xT_sb = hpool.tile([INF, cfg.n_own], F16, tag="hx", name="xT")
        nc.sync.dma_start(xT_sb[:], xT_d[:, :])
        tc.no_sync_barrier()

        # ---- internal DRAM ----------------------------------------------
        slabs = [dram.tile([cfg.n_own, HID], F16, tag=f"slab{l}", name=f"slab{l}")
                 for l in range(3)]
        tbls = [dram.tile([NC * cfg.n_own, HID], F16, tag=f"tbl{l}",
                          name=f"tbl{l}", addr_space="Shared")
                for l in range(3)]

        # h^T slabs stay in SBUF between layers (allocated lazily per layer)
        hts = [None, None]

        # ---- P0: m1 = x @ Wl1 (row-major slab) --------------------------
        for t in range(NT):
            m_ps = ps_m.tile([P, HID], F32, tag="mps")
            nc.tensor.matmul(
                out=m_ps[:], lhsT=xT_sb[:, t * P:(t + 1) * P], rhs=Wl1_sb[:],
                start=True, stop=True,
            )
            m_sb = sb2.tile([P, HID], F16, tag="msb")
            nc.vector.tensor_copy(m_sb[:], m_ps[:])
            nc.sync.dma_start(slabs[0][t * P:(t + 1) * P, :], m_sb[:])

        if "noag" not in ABLATE:
            nc.gpsimd.collective_compute(
                "AllGather", mybir.AluOpType.bypass, replica_groups=rg,
                ins=[slabs[0].opt()], outs=[tbls[0].opt()],
            )
        tc.no_sync_barrier()

        # ---- layers ------------------------------------------------------
        for layer in range(3):
            tbl = tbls[layer]
            Wr_sb = (Wr1_sb, Wr2_sb, Wr3_sb)[layer]
            bl_sb = (bl1_sb, bl2_sb, bl3_sb)[layer]
            Wl_next = (Wl2_sb, Wl3_sb, None)[layer]
            if layer < 2:
                hts[layer] = hpool.tile([HID, cfg.n_own], F16, tag="hx",
                                        name=f"ht{layer}")
            h_prev = xT_sb if layer == 0 else hts[layer - 1]
            h_new = hts[layer] if layer < 2 else None

            if layer == 2:
                poolT_ps = pool_ps.tile([HID, GPC], F32, tag="pool")

            for b, (st0, nb_st) in enumerate(batches):
                bc0 = int(batch_col0[b])
                g_t = gbuf.tile([P, max_batch_cols * P], F16, tag="g")
                rel = 0
                for c in range(CH):
                    ncols = int(call_cols[b, c])
                    if ncols == 0 or "nogather" in ABLATE:
                        rel += ncols
                        continue
                    col0 = bc0 + rel
                    nidx = ncols * P
                    nc.gpsimd.dma_gather(
                        out_ap=g_t[:, rel * P:(rel + ncols) * P].rearrange(
                            "p (t e) -> p t e", e=HID),
                        in_ap=tbl[c * CHROWS:(c + 1) * CHROWS, :],
                        idxs_ap=idx_sb[:, col0 * 8:(col0 + ncols) * 8],
                        num_idxs=nidx,
                        num_idxs_reg=nidx,
                        elem_size=HID,
                        single_packet=False,
                    )
                    rel += ncols

                for s in range(nb_st):
                    st = st0 + s
                    out_ps = ps.tile([HID, ST * P], F32, tag="outT")
                    first = True
                    if "nomaskmm" not in ABLATE:
                        for c in range(CH):
                            nsub = int(sub[st, c])
                            if nsub == 0:
                                continue
                            # one batched 0/1 mask build for all sub-tiles of
                            # this (supertile, chunk): (iota == dstrel)
                            k0 = int(subid[st, c])
                            mask_t = mpool.tile([P, max_stc_sub * ST * P], F16,
                                                tag="mask")
                            nc.vector.tensor_tensor(
                                out=mask_t[:, :nsub * ST * P].rearrange(
                                    "p (n s) -> p n s", s=ST * P),
                                in0=iota_f[:].rearrange(
                                    "p (o s) -> p o s", o=1).broadcast_to(
                                    [P, nsub, ST * P]),
                                in1=dstrel_sb[:, k0:k0 + nsub].rearrange(
                                    "p (n o) -> p n o", o=1).broadcast_to(
                                    [P, nsub, ST * P]),
                                op=mybir.AluOpType.is_equal,
                            )
                            for j in range(nsub):
                                k_rel = int(gcol[st, c]) - bc0 + j
                                kg = int(gcol[st, c]) + j
                                # fold per-edge weight into the gathered rows
                                # on the idle Scalar engine (per-partition
                                # scale)
                                gw = sb2.tile([P, P], F16, tag="gw")
                                nc.scalar.activation(
                                    gw[:],
                                    g_t[:, k_rel * P:(k_rel + 1) * P],
                                    mybir.ActivationFunctionType.Copy,
                                    scale=wg_sb[:, kg:kg + 1],
                                )
                                nc.tensor.matmul(
                                    out=out_ps[:],
                                    lhsT=gw[:],
                                    rhs=mask_t[:, j * ST * P:(j + 1) * ST * P],
                                    start=first, stop=False,
                                )
                                first = False
                    # self terms into the 4 column slices
                    for ti in range(ST):
                        t = st * ST + ti
                        nc.tensor.matmul(
                            out=out_ps[:, ti * P:(ti + 1) * P],
                            lhsT=Wr_sb[:], rhs=h_prev[:, t * P:(t + 1) * P],
                            start=first, stop=True,
                        )
                    first = False

                    if layer < 2:
                        for ti in range(ST):
                            t = st * ST + ti
                            nc.scalar.activation(
                                h_new[:, t * P:(t + 1) * P],
                                out_ps[:, ti * P:(ti + 1) * P],
                                mybir.ActivationFunctionType.Relu,
                                bias=bl_sb[:, :1],
                            )
                            m_ps = ps_m.tile([P, HID], F32, tag="mps")
                            nc.tensor.matmul(
                                out=m_ps[:], lhsT=h_new[:, t * P:(t + 1) * P],
                                rhs=Wl_next[:], start=True, stop=True,
                            )
                            m_sb = sb2.tile([P, HID], F16, tag="msb")
                            nc.vector.tensor_copy(m_sb[:], m_ps[:])
                            nc.sync.dma_start(
                                slabs[layer + 1][t * P:(t + 1) * P, :], m_sb[:])
                    else:
                        h3_sb = sb2.tile([HID, ST * P], F16, tag="h3")
                        nc.vector.tensor_scalar(
                            out=h3_sb[:], in0=out_ps[:],
                            scalar1=bl_sb[:, :1], scalar2=None,
                            op0=mybir.AluOpType.add,
                        )
                        for ti in range(ST):
                            t = st * ST + ti
                            h3rm_ps = ps_m.tile([P, HID], F16, tag="h3rm")
                            nc.tensor.transpose(
                                h3rm_ps[:], h3_sb[:, ti * P:(ti + 1) * P],
                                ident16[:])
                            h3rm_sb = sb2.tile([P, HID], F16, tag="h3rmsb")
                            nc.vector.tensor_copy(h3rm_sb[:], h3rm_ps[:])
                            nc.tensor.matmul(
                                out=poolT_ps[:], lhsT=h3rm_sb[:],
                                rhs=gmask_sb[:, t * GPC:(t + 1) * GPC],
                                start=(t == 0), stop=(t == NT - 1),
                            )
                if "nobarrier" not in ABLATE:
                    tc.no_sync_barrier()

            if layer < 2 and "noag" not in ABLATE:
                nc.gpsimd.collective_compute(
                    "AllGather", mybir.AluOpType.bypass, replica_groups=rg,
                    ins=[slabs[layer + 1].opt()], outs=[tbls[layer + 1].opt()],
                )
                tc.no_sync_barrier()

        # ---- head --------------------------------------------------------
        poolT_sb = sb.tile([HID, GPC], F16)
        nc.vector.tensor_copy(poolT_sb[:], poolT_ps[:])
        fin_ps = pool_ps.tile([cfg.num_classes, GPC], F32, tag="fin")
        nc.tensor.matmul(
            out=fin_ps[:], lhsT=Wlin_sb[:], rhs=poolT_sb[:], start=True, stop=True,
        )
        fin_sb = sb.tile([cfg.num_classes, GPC], F32)
        nc.vector.tensor_scalar(
            out=fin_sb[:], in0=fin_ps[:],
            scalar1=blin_sb[:, :1], scalar2=None,
            op0=mybir.AluOpType.add,
        )
        nc.sync.dma_start(out_d[:, :], fin_sb[:])

    nc.compile()
    return nc


# --------------------------------------------------------------------------
# Host-side preprocessing
# --------------------------------------------------------------------------

def preprocess(x, edge_index, batch, cfg_overrides=None):
    num_nodes = x.shape[0]
    in_feat = x.shape[1]
    num_edges = edge_index.shape[1]
    batch = np.asarray(batch, dtype=np.int64)
    src_all = np.asarray(edge_index[0], dtype=np.int64)
    dst_all = np.asarray(edge_index[1], dtype=np.int64)
    n_cores = 8
    num_graphs = int(cfg_overrides.get("num_graphs")) if cfg_overrides and "num_graphs" in cfg_overrides else 512
    gpc = num_graphs // n_cores

    bounds = np.searchsorted(batch, np.arange(n_cores + 1) * gpc)
    nl = bounds[1:] - bounds[:-1]
    blk = ST * P
    n_own = int(-(-int(nl.max()) // blk) * blk)
    assert 2 * n_own <= 32767, "int16 chunk limit"
    chrows = 2 * n_own
    nt = n_own // P
    n_st = nt // ST

    deg = np.bincount(dst_all, minlength=num_nodes)
    w_all = np.zeros(num_edges, np.float32)
    nz = deg[dst_all] > 0
    w_all[nz] = 1.0 / deg[dst_all[nz]]

    owner_d = (batch[dst_all] // gpc).astype(np.int64)
    owner_s = (batch[src_all] // gpc).astype(np.int64)
    src_row = (owner_s * n_own + (src_all - bounds[owner_s])).astype(np.int64)
    chunk = src_row // chrows
    src_rel = (src_row - chunk * chrows).astype(np.int16)
    ld = (dst_all - bounds[owner_d]).astype(np.int64)
    tile_of = ld // P
    st_of = tile_of // ST

    # exact per-(core, st, chunk) counts; program uses max over cores
    gkey = (owner_d * n_st + st_of) * CH + chunk
    ngroups = n_cores * n_st * CH
    gcounts = np.bincount(gkey, minlength=ngroups)
    cnt3 = gcounts.reshape(n_cores, n_st, CH)
    sub_max = -(-cnt3.max(axis=0) // P)          # [n_st, CH] ceil
    sub_max_flat = tuple(int(v) for v in sub_max.reshape(-1))

    cfg = Cfg(
        n_cores=n_cores, num_nodes=num_nodes, num_edges=num_edges,
        in_feat=in_feat, hidden=128, num_graphs=num_graphs,
        num_classes=2, n_own=n_own, sub_max=sub_max_flat, gpc=gpc,
    )
    gcol, subid, call_cols, batch_col0 = _layout(cfg)
    TOTCOL = cfg.tot_sub
    e_proc = TOTCOL * P

    # slot assignment: rank within (core, st, chunk) group
    order = np.argsort(gkey, kind="stable")
    gk_sorted = gkey[order]
    group_start = np.zeros(ngroups, np.int64)
    group_start[1:] = np.cumsum(gcounts)[:-1]
    rank = np.arange(num_edges) - group_start[gk_sorted]
    st_s = (gk_sorted // CH) % n_st
    c_s = gk_sorted % CH
    core_s = gk_sorted // (n_st * CH)
    col = gcol[st_s, c_s] + rank // P
    slot = col * P + rank % P
    row = rank % P

    idx_arr = np.zeros((n_cores, e_proc), np.int16)
    idx_arr[core_s, slot] = src_rel[order]

    # dstrel (subid order): dst position within the 512-wide supertile;
    # padding slots -1000 so is_equal never fires.
    # wg (gcol order): per-slot weight folded into the gathered rows.
    sub_id_edge = subid[st_s, c_s] + rank // P
    dstrel_arr = np.full((n_cores, P, TOTCOL), -1000.0, np.float16)
    w_arr = np.zeros((n_cores, P, TOTCOL), np.float32)
    dpos = (tile_of[order] % ST) * P + (ld[order] - tile_of[order] * P)
    dstrel_arr[core_s, row, sub_id_edge] = dpos.astype(np.float16)
    w_arr[core_s, row, col] = w_all[order]

    def to_i16(a):
        band = a.reshape(e_proc // 16, 16).T
        return np.ascontiguousarray(np.tile(band, (8, 1)))

    gsizes = np.bincount(batch, minlength=num_graphs).astype(np.float32)
    per_core = []
    for c in range(n_cores):
        n0, n1 = int(bounds[c]), int(bounds[c + 1])
        xT = np.zeros((in_feat, n_own), np.float16)
        xT[:, : n1 - n0] = x[n0:n1].T.astype(np.float16)
        # gmask [128, NT*GPC]: node tile t, graph col g -> (batch==g)/graphsize
        gm = np.zeros((n_own, gpc), np.float16)
        loc = np.arange(n1 - n0)
        grel = (batch[n0:n1] - c * gpc).astype(np.int64)
        gs = gsizes[batch[n0:n1]]
        val = np.zeros(n1 - n0, np.float32)
        val[gs > 0] = 1.0 / gs[gs > 0]
        gm[loc, grel] = val.astype(np.float16)
        gmask = np.ascontiguousarray(
            gm.reshape(nt, P, gpc).transpose(1, 0, 2).reshape(P, nt * gpc))

        per_core.append(dict(
            xT=xT,
            eidx=to_i16(idx_arr[c]),
            edstrel=np.ascontiguousarray(dstrel_arr[c]),
            ewg=np.ascontiguousarray(w_arr[c]),
            gmask=gmask,
        ))

    return cfg, per_core


def make_in_maps(cfg, per_core, weights):
    wmap = {}
    for k in ("Wl1", "Wr1", "Wl2", "Wr2", "Wl3", "Wr3", "Wlin"):
        wmap[k] = np.ascontiguousarray(weights[k].astype(np.float16))
    for k in ("bl1", "bl2", "bl3", "blin"):
        wmap[k] = np.ascontiguousarray(weights[k].astype(np.float32).reshape(-1, 1))
    in_maps = []
    for c in range(cfg.n_cores):
        m = dict(per_core[c])
        m.update(wmap)
        in_maps.append(m)
    return in_maps


_PROGRAM_CACHE = {}


def kernel(x, edge_index, batch,
           Wl1, bl1, Wr1, Wl2, bl2, Wr2, Wl3, bl3, Wr3, Wlin, blin):
    x = np.asarray(x)
    cfg, per_core = preprocess(np.asarray(x, np.float32),
                               np.asarray(edge_index), np.asarray(batch))
    weights = dict(Wl1=np.asarray(Wl1), bl1=np.asarray(bl1), Wr1=np.asarray(Wr1),
                   Wl2=np.asarray(Wl2), bl2=np.asarray(bl2), Wr2=np.asarray(Wr2),
                   Wl3=np.asarray(Wl3), bl3=np.asarray(bl3), Wr3=np.asarray(Wr3),
                   Wlin=np.asarray(Wlin), blin=np.asarray(blin))
    in_maps = make_in_maps(cfg, per_core, weights)

    key = (cfg.n_own, cfg.sub_max, cfg.in_feat, cfg.num_graphs)
    if key not in _PROGRAM_CACHE:
        _PROGRAM_CACHE[key] = build_program(cfg)
    nc = _PROGRAM_CACHE[key]

    res = bass_utils.run_bass_kernel_spmd(
        nc, in_maps, core_ids=list(range(cfg.n_cores)),
    )
    out = np.empty((cfg.num_graphs, cfg.num_classes), np.float32)
    for c in range(cfg.n_cores):
        out[c * cfg.gpc:(c + 1) * cfg.gpc, :] = res.results[c]["out"].T
    return out


# revision 35
# speedup vs baseline: 1.8669x; 1.4403x over previous
"""Distributed GraphSAGE kernel for Trainium2 (8 NeuronCores, Bass/Tile). v2

Takes FULL inputs (same keys as setup_inputs()), shards by graph id across 8
cores, runs a single SPMD Bass program (3 SAGE layers + global mean pool +
linear head) with inter-layer AllGathers, returns the FULL [512, 2] output.

v2 changes vs v1:
  - one-hot scatter masks (iota==dst)*1/deg are PRECOMPUTED ON HOST and
    streamed from DRAM as matmul rhs operands (v1 built them per-subtile on
    DVE: ~3.8ms of vector-engine time, the top bottleneck)
  - aggregation runs on 512-wide supertiles (4 node tiles per PSUM tile) with
    EXACT per-(supertile,chunk) edge sub-tile counts (max over the 8 cores so
    the SPMD program is shared); v1 used a global worst-case te_c budget
    (padding 153k slots/core/layer -> 111k)
  - h^T slabs live in SBUF (no DRAM round trip between layers)
  - graph-pool masks precomputed on host as well
  - per-batch (not per-tile) scheduler barriers

Algorithm per core (nodes sharded by graph; batch sorted so each core owns a
contiguous node range; edges assigned to the core owning their dst):
  - table TBL_l holds m_l = h_{l-1} @ Wl_l for ALL nodes (fp16, allgathered);
    TBL split into 4 row-chunks of 2*n_own rows for int16 dma_gather indices
  - per gather batch (NBS supertiles): 4 dma_gather calls (one per chunk)
    with exact slot counts; per supertile: mask-matmuls accumulate
    (mean_agg @ Wl_l)^T into a [128, 512] PSUM tile, self terms
    Wr_l^T @ h^T accumulate into the 4 column slices
  - relu+bias on ACT -> h_l^T slab (SBUF); m_{l+1} matmul per tile -> slab ->
    AllGather
  - layer 3: bias on DVE, transpose tiles via TensorE, pool with precomputed
    per-graph masks, then Wlin matmul + bias
"""
import sys
import os

sys.path.insert(0, "/opt/trn_rl_repo")

import numpy as np
from contextlib import ExitStack
from dataclasses import dataclass

from concourse import bass, mybir, tile, bacc
from concourse import bass_utils
from concourse.masks import make_identity

P = 128
CH = 4              # table row chunks (int16 index limit)
ST = 4              # node tiles per supertile (512-wide PSUM)
BATCH_ST = 3        # supertiles per gather batch (last batch may be smaller)
F16 = mybir.dt.float16
F32 = mybir.dt.float32
I16 = mybir.dt.int16

ABLATE = frozenset()


@dataclass(frozen=True)
class Cfg:
    n_cores: int
    num_nodes: int
    num_edges: int
    in_feat: int
    hidden: int
    num_graphs: int
    num_classes: int
    n_own: int             # padded nodes per core (multiple of ST*128)
    sub_max: tuple         # flat tuple, sub_max[st*CH+c] subtiles per (st,chunk)
    gpc: int               # graphs per core

    @property
    def nt(self):
        return self.n_own // P

    @property
    def n_st(self):
        return self.nt // ST

    @property
    def batches(self):
        """List of (st0, n_st_in_batch)."""
        out = []
        st = 0
        while st < self.n_st:
            n = min(BATCH_ST, self.n_st - st)
            out.append((st, n))
            st += n
        return out

    @property
    def sub_arr(self):
        return np.asarray(self.sub_max, np.int64).reshape(self.n_st, CH)

    @property
    def tot_sub(self):
        return int(self.sub_arr.sum())


def _layout(cfg: Cfg):
    """Static layout tables shared by host preprocessing and program build.

    Returns:
      gcol   [n_st, CH]: first g_t column (within the layer-global column
             space, order (batch, chunk, st, j)) of each (st, chunk) group
      subid  [n_st, CH]: first mask sub-tile id (order (batch, st, chunk, j))
      call_cols [n_batch, CH]: columns per dma_gather call
      batch_col0 [n_batch]: first global column of each batch
    """
    sub = cfg.sub_arr
    batches = cfg.batches
    n_b = len(batches)
    gcol = np.zeros((cfg.n_st, CH), np.int64)
    subid = np.zeros((cfg.n_st, CH), np.int64)
    call_cols = np.zeros((n_b, CH), np.int64)
    batch_col0 = np.zeros(n_b, np.int64)
    col = 0
    for b, (st0, nb) in enumerate(batches):
        batch_col0[b] = col
        for c in range(CH):
            for s in range(nb):
                st = st0 + s
                gcol[st, c] = col
                col += sub[st, c]
            call_cols[b, c] = int(sub[st0:st0 + nb, c].sum())
    sid = 0
    for st in range(cfg.n_st):
        for c in range(CH):
            subid[st, c] = sid
            sid += sub[st, c]
    return gcol, subid, call_cols, batch_col0


def build_program(cfg: Cfg):
    nc = bacc.Bacc(
        "TRN2",
        target_bir_lowering=False,
        debug=False,
        num_devices=cfg.n_cores,
        num_swdge_queues=2,
    )

    NT = cfg.nt
    HID = cfg.hidden
    INF = cfg.in_feat
    GPC = cfg.gpc
    NC = cfg.n_cores
    CHROWS = 2 * cfg.n_own
    sub = cfg.sub_arr
    batches = cfg.batches
    n_b = len(batches)
    gcol, subid, call_cols, batch_col0 = _layout(cfg)
    TOTCOL = int(sub.sum())
    max_batch_cols = int(max(
        (batch_col0[b + 1] if b + 1 < n_b else TOTCOL) - batch_col0[b]
        for b in range(n_b)
    ))
    max_stc_sub = int(sub.max())

    # ---- I/O -------------------------------------------------------------
    xT_d = nc.dram_tensor("xT", [INF, cfg.n_own], F16, kind="ExternalInput")
    idx_d = nc.dram_tensor("eidx", [P, TOTCOL * 8], I16, kind="ExternalInput")
    # dstrel in mask-subtile (subid) order; per-slot weight in g_t (gcol) order
    dstrel_d = nc.dram_tensor("edstrel", [P, TOTCOL], F16, kind="ExternalInput")
    wg_d = nc.dram_tensor("ewg", [P, TOTCOL], F32, kind="ExternalInput")
    gmask_d = nc.dram_tensor("gmask", [P, NT * GPC], F16, kind="ExternalInput")
    Wl1_d = nc.dram_tensor("Wl1", [INF, HID], F16, kind="ExternalInput")
    Wr1_d = nc.dram_tensor("Wr1", [INF, HID], F16, kind="ExternalInput")
    Wl2_d = nc.dram_tensor("Wl2", [HID, HID], F16, kind="ExternalInput")
    Wr2_d = nc.dram_tensor("Wr2", [HID, HID], F16, kind="ExternalInput")
    Wl3_d = nc.dram_tensor("Wl3", [HID, HID], F16, kind="ExternalInput")
    Wr3_d = nc.dram_tensor("Wr3", [HID, HID], F16, kind="ExternalInput")
    Wlin_d = nc.dram_tensor("Wlin", [HID, cfg.num_classes], F16, kind="ExternalInput")
    bl1_d = nc.dram_tensor("bl1", [HID, 1], F32, kind="ExternalInput")
    bl2_d = nc.dram_tensor("bl2", [HID, 1], F32, kind="ExternalInput")
    bl3_d = nc.dram_tensor("bl3", [HID, 1], F32, kind="ExternalInput")
    blin_d = nc.dram_tensor("blin", [cfg.num_classes, 1], F32, kind="ExternalInput")
    out_d = nc.dram_tensor("out", [cfg.num_classes, GPC], F32, kind="ExternalOutput")

    rg = [list(range(NC))]

    with tile.TileContext(nc) as tc, ExitStack() as ctx:
        sb = ctx.enter_context(tc.tile_pool(name="sb", bufs=1))
        hpool = ctx.enter_context(tc.tile_pool(name="hp", bufs=2))
        sb2 = ctx.enter_context(tc.tile_pool(name="sb2", bufs=4))
        mpool = ctx.enter_context(tc.tile_pool(name="mp", bufs=2))
        gbuf = ctx.enter_context(tc.tile_pool(name="gbuf", bufs=2))
        ps = ctx.enter_context(tc.tile_pool(name="ps", bufs=2, space="PSUM"))
        ps_m = ctx.enter_context(tc.tile_pool(name="psm", bufs=2, space="PSUM"))
        pool_ps = ctx.enter_context(tc.tile_pool(name="pps", bufs=1, space="PSUM"))
        dram = ctx.enter_context(tc.tile_pool(name="dram", bufs=1, space="DRAM"))

        # ---- static SBUF state ------------------------------------------
        ident16 = sb.tile([P, P], F16)
        make_identity(nc, ident16[:])

        # iota over the 512 supertile columns, fp16 (exact for 0..511)
        iota_i = sb.tile([P, ST * P], mybir.dt.int32)
        nc.gpsimd.iota(iota_i[:], pattern=[[1, ST * P]], base=0,
                       channel_multiplier=0)
        iota_f = sb.tile([P, ST * P], F16)
        nc.vector.tensor_copy(iota_f[:], iota_i[:])

        idx_sb = sb.tile([P, TOTCOL * 8], I16)
        nc.sync.dma_start(idx_sb[:], idx_d[:, :])
        dstrel_sb = sb.tile([P, TOTCOL], F16)
        nc.sync.dma_start(dstrel_sb[:], dstrel_d[:, :])
        wg_sb = sb.tile([P, TOTCOL], F32)
        nc.sync.dma_start(wg_sb[:], wg_d[:, :])
        gmask_sb = sb.tile([P, NT * GPC], F16)
        nc.sync.dma_start(gmask_sb[:], gmask_d[:, :])

        def load_w(d, p_, f_, nm):
            t = sb.tile([p_, f_], F16, name=nm, tag=nm)
            nc.sync.dma_start(t[:], d[:, :])
            return t

        Wl1_sb = load_w(Wl1_d, INF, HID, "wl1s")
        Wr1_sb = load_w(Wr1_d, INF, HID, "wr1s")
        Wl2_sb = load_w(Wl2_d, HID, HID, "wl2s")
        Wr2_sb = load_w(Wr2_d, HID, HID, "wr2s")
        Wl3_sb = load_w(Wl3_d, HID, HID, "wl3s")
        Wr3_sb = load_w(Wr3_d, HID, HID, "wr3s")
        Wlin_sb = load_w(Wlin_d, HID, cfg.num_classes, "wlins")
        bl1_sb = sb.tile([HID, 1], F32)
        nc.sync.dma_start(bl1_sb[:], bl1_d[:, :])
        bl2_sb = sb.tile([HID, 1], F32)
        nc.sync.dma_start(bl2_sb[:], bl2_d[:, :])
        bl3_sb = sb.tile([HID, 1], F32)
        nc.sync.dma_start(bl3_sb[:], bl3_d[:, :])
        blin_sb = sb.tile([cfg.num_classes, 1], F32)
        nc.sync.dma_start(blin_sb[:], blin_d[:, :])

        # xT and the two h^T slabs share one 2-slot tag: ht1 reuses xT's slot
        # once layer 1 (the last xT reader) is done
        xT_sb = hpool.tile([INF, cfg.n_own], F16, tag="hx", name="xT")
        nc.sync.dma_start(xT_sb[:], xT_d[:, :])
        tc.no_sync_barrier()

        # ---- internal DRAM ----------------------------------------------
        slabs = [dram.tile([cfg.n_own, HID], F16, tag=f"slab{l}", name=f"slab{l}")
                 for l in range(3)]
        tbls = [dram.tile([NC * cfg.n_own, HID], F16, tag=f"tbl{l}",
                          name=f"tbl{l}", addr_space="Shared")
                for l in range(3)]

        # h^T slabs stay in SBUF between layers (allocated lazily per layer)
        hts = [None, None]

        # ---- P0: m1 = x @ Wl1 (row-major slab) --------------------------
        for t in range(NT):
            m_ps = ps_m.tile([P, HID], F32, tag="mps")
            nc.tensor.matmul(
                out=m_ps[:], lhsT=xT_sb[:, t * P:(t + 1) * P], rhs=Wl1_sb[:],
                start=True, stop=True,
            )
            m_sb = sb2.tile([P, HID], F16, tag="msb")
            nc.vector.tensor_copy(m_sb[:], m_ps[:])
            nc.sync.dma_start(slabs[0][t * P:(t + 1) * P, :], m_sb[:])

        if "noag" not in ABLATE:
            nc.gpsimd.collective_compute(
                "AllGather", mybir.AluOpType.bypass, replica_groups=rg,
                ins=[slabs[0].opt()], outs=[tbls[0].opt()],
            )
        tc.no_sync_barrier()

        # ---- layers ------------------------------------------------------
        for layer in range(3):
            tbl = tbls[layer]
            Wr_sb = (Wr1_sb, Wr2_sb, Wr3_sb)[layer]
            bl_sb = (bl1_sb, bl2_sb, bl3_sb)[layer]
            Wl_next = (Wl2_sb, Wl3_sb, None)[layer]
            if layer < 2:
                hts[layer] = hpool.tile([HID, cfg.n_own], F16, tag="hx",
                                        name=f"ht{layer}")
            h_prev = xT_sb if layer == 0 else hts[layer - 1]
            h_new = hts[layer] if layer < 2 else None

            if layer == 2:
                poolT_ps = pool_ps.tile([HID, GPC], F32, tag="pool")

            for b, (st0, nb_st) in enumerate(batches):
                bc0 = int(batch_col0[b])
                g_t = gbuf.tile([P, max_batch_cols * P], F16, tag="g")
                rel = 0
                for c in range(CH):
                    ncols = int(call_cols[b, c])
                    if ncols == 0 or "nogather" in ABLATE:
                        rel += ncols
                        continue
                    col0 = bc0 + rel
                    nidx = ncols * P
                    nc.gpsimd.dma_gather(
                        out_ap=g_t[:, rel * P:(rel + ncols) * P].rearrange(
                            "p (t e) -> p t e", e=HID),
                        in_ap=tbl[c * CHROWS:(c + 1) * CHROWS, :],
                        idxs_ap=idx_sb[:, col0 * 8:(col0 + ncols) * 8],
                        num_idxs=nidx,
                        num_idxs_reg=nidx,
                        elem_size=HID,
                        single_packet=False,
                        queue_num=c % 2,
                    )
                    rel += ncols

                for s in range(nb_st):
                    st = st0 + s
                    out_ps = ps.tile([HID, ST * P], F32, tag="outT")
                    first = True
                    if "nomaskmm" not in ABLATE:
                        for c in range(CH):
                            nsub = int(sub[st, c])
                            if nsub == 0:
                                continue
                            # one batched 0/1 mask build for all sub-tiles of
                            # this (supertile, chunk): (iota == dstrel)
                            k0 = int(subid[st, c])
                            mask_t = mpool.tile([P, max_stc_sub * ST * P], F16,
                                                tag="mask")
                            nc.vector.tensor_tensor(
                                out=mask_t[:, :nsub * ST * P].rearrange(
                                    "p (n s) -> p n s", s=ST * P),
                                in0=iota_f[:].rearrange(
                                    "p (o s) -> p o s", o=1).broadcast_to(
                                    [P, nsub, ST * P]),
                                in1=dstrel_sb[:, k0:k0 + nsub].rearrange(
                                    "p (n o) -> p n o", o=1).broadcast_to(
                                    [P, nsub, ST * P]),
                                op=mybir.AluOpType.is_equal,
                            )
                            for j in range(nsub):
                                k_rel = int(gcol[st, c]) - bc0 + j
                                kg = int(gcol[st, c]) + j
                                # fold per-edge weight into the gathered rows
                                # on the idle Scalar engine (per-partition
                                # scale)
                                gw = sb2.tile([P, P], F16, tag="gw")
                                nc.scalar.activation(
                                    gw[:],
                                    g_t[:, k_rel * P:(k_rel + 1) * P],
                                    mybir.ActivationFunctionType.Copy,
                                    scale=wg_sb[:, kg:kg + 1],
                                )
                                nc.tensor.matmul(
                                    out=out_ps[:],
                                    lhsT=gw[:],
                                    rhs=mask_t[:, j * ST * P:(j + 1) * ST * P],
                                    start=first, stop=False,
                                )
                                first = False
                    # self terms into the 4 column slices
                    for ti in range(ST):
                        t = st * ST + ti
                        nc.tensor.matmul(
                            out=out_ps[:, ti * P:(ti + 1) * P],
                            lhsT=Wr_sb[:], rhs=h_prev[:, t * P:(t + 1) * P],
                            start=first, stop=True,
                        )
                    first = False

                    if layer < 2:
                        for ti in range(ST):
                            t = st * ST + ti
                            nc.scalar.activation(
                                h_new[:, t * P:(t + 1) * P],
                                out_ps[:, ti * P:(ti + 1) * P],
                                mybir.ActivationFunctionType.Relu,
                                bias=bl_sb[:, :1],
                            )
                            m_ps = ps_m.tile([P, HID], F32, tag="mps")
                            nc.tensor.matmul(
                                out=m_ps[:], lhsT=h_new[:, t * P:(t + 1) * P],
                                rhs=Wl_next[:], start=True, stop=True,
                            )
                            m_sb = sb2.tile([P, HID], F16, tag="msb")
                            nc.vector.tensor_copy(m_sb[:], m_ps[:])
                            nc.sync.dma_start(
                                slabs[layer + 1][t * P:(t + 1) * P, :], m_sb[:])
                    else:
                        h3_sb = sb2.tile([HID, ST * P], F16, tag="h3")
                        nc.vector.tensor_scalar(
                            out=h3_sb[:], in0=out_ps[:],
                            scalar1=bl_sb[:, :1], scalar2=None,
                            op0=mybir.AluOpType.add,
                        )
                        for ti in range(ST):
                            t = st * ST + ti
                            h3rm_ps = ps_m.tile([P, HID], F16, tag="h3rm")
                            nc.tensor.transpose(
                                h3rm_ps[:], h3_sb[:, ti * P:(ti + 1) * P],
                                ident16[:])
                            h3rm_sb = sb2.tile([P, HID], F16, tag="h3rmsb")
                            nc.vector.tensor_copy(h3rm_sb[:], h3rm_ps[:])
                            nc.tensor.matmul(
                                out=poolT_ps[:], lhsT=h3rm_sb[:],
                                rhs=gmask_sb[:, t * GPC:(t + 1) * GPC],
                                start=(t == 0), stop=(t == NT - 1),
                            )
                if "nobarrier" not in ABLATE:
                    tc.no_sync_barrier()

            if layer < 2 and "noag" not in ABLATE:
                nc.gpsimd.collective_compute(
                    "AllGather", mybir.AluOpType.bypass, replica_groups=rg,
                    ins=[slabs[layer + 1].opt()], outs=[tbls[layer + 1].opt()],
                )
                tc.no_sync_barrier()

        # ---- head --------------------------------------------------------
        poolT_sb = sb.tile([HID, GPC], F16)
        nc.vector.tensor_copy(poolT_sb[:], poolT_ps[:])
        fin_ps = pool_ps.tile([cfg.num_classes, GPC], F32, tag="fin")
        nc.tensor.matmul(
            out=fin_ps[:], lhsT=Wlin_sb[:], rhs=poolT_sb[:], start=True, stop=True,
        )
        fin_sb = sb.tile([cfg.num_classes, GPC], F32)
        nc.vector.tensor_scalar(
            out=fin_sb[:], in0=fin_ps[:],
            scalar1=blin_sb[:, :1], scalar2=None,
            op0=mybir.AluOpType.add,
        )
        nc.sync.dma_start(out_d[:, :], fin_sb[:])

    nc.compile()
    return nc


# --------------------------------------------------------------------------
# Host-side preprocessing
# --------------------------------------------------------------------------

def preprocess(x, edge_index, batch, cfg_overrides=None):
    num_nodes = x.shape[0]
    in_feat = x.shape[1]
    num_edges = edge_index.shape[1]
    batch = np.asarray(batch, dtype=np.int64)
    src_all = np.asarray(edge_index[0], dtype=np.int64)
    dst_all = np.asarray(edge_index[1], dtype=np.int64)
    n_cores = 8
    num_graphs = int(cfg_overrides.get("num_graphs")) if cfg_overrides and "num_graphs" in cfg_overrides else 512
    gpc = num_graphs // n_cores

    bounds = np.searchsorted(batch, np.arange(n_cores + 1) * gpc)
    nl = bounds[1:] - bounds[:-1]
    blk = ST * P
    n_own = int(-(-int(nl.max()) // blk) * blk)
    assert 2 * n_own <= 32767, "int16 chunk limit"
    chrows = 2 * n_own
    nt = n_own // P
    n_st = nt // ST

    deg = np.bincount(dst_all, minlength=num_nodes)
    w_all = np.zeros(num_edges, np.float32)
    nz = deg[dst_all] > 0
    w_all[nz] = 1.0 / deg[dst_all[nz]]

    owner_d = (batch[dst_all] // gpc).astype(np.int64)
    owner_s = (batch[src_all] // gpc).astype(np.int64)
    src_row = (owner_s * n_own + (src_all - bounds[owner_s])).astype(np.int64)
    chunk = src_row // chrows
    src_rel = (src_row - chunk * chrows).astype(np.int16)
    ld = (dst_all - bounds[owner_d]).astype(np.int64)
    tile_of = ld // P
    st_of = tile_of // ST

    # exact per-(core, st, chunk) counts; program uses max over cores
    gkey = (owner_d * n_st + st_of) * CH + chunk
    ngroups = n_cores * n_st * CH
    gcounts = np.bincount(gkey, minlength=ngroups)
    cnt3 = gcounts.reshape(n_cores, n_st, CH)
    sub_max = -(-cnt3.max(axis=0) // P)          # [n_st, CH] ceil
    sub_max_flat = tuple(int(v) for v in sub_max.reshape(-1))

    cfg = Cfg(
        n_cores=n_cores, num_nodes=num_nodes, num_edges=num_edges,
        in_feat=in_feat, hidden=128, num_graphs=num_graphs,
        num_classes=2, n_own=n_own, sub_max=sub_max_flat, gpc=gpc,
    )
    gcol, subid, call_cols, batch_col0 = _layout(cfg)
    TOTCOL = cfg.tot_sub
    e_proc = TOTCOL * P

    # slot assignment: rank within (core, st, chunk) group
    order = np.argsort(gkey, kind="stable")
    gk_sorted = gkey[order]
    group_start = np.zeros(ngroups, np.int64)
    group_start[1:] = np.cumsum(gcounts)[:-1]
    rank = np.arange(num_edges) - group_start[gk_sorted]
    st_s = (gk_sorted // CH) % n_st
    c_s = gk_sorted % CH
    core_s = gk_sorted // (n_st * CH)
    col = gcol[st_s, c_s] + rank // P
    slot = col * P + rank % P
    row = rank % P

    idx_arr = np.zeros((n_cores, e_proc), np.int16)
    idx_arr[core_s, slot] = src_rel[order]

    # dstrel (subid order): dst position within the 512-wide supertile;
    # padding slots -1000 so is_equal never fires.
    # wg (gcol order): per-slot weight folded into the gathered rows.
    sub_id_edge = subid[st_s, c_s] + rank // P
    dstrel_arr = np.full((n_cores, P, TOTCOL), -1000.0, np.float16)
    w_arr = np.zeros((n_cores, P, TOTCOL), np.float32)
    dpos = (tile_of[order] % ST) * P + (ld[order] - tile_of[order] * P)
    dstrel_arr[core_s, row, sub_id_edge] = dpos.astype(np.float16)
    w_arr[core_s, row, col] = w_all[order]

    def to_i16(a):
        band = a.reshape(e_proc // 16, 16).T
        return np.ascontiguousarray(np.tile(band, (8, 1)))

    gsizes = np.bincount(batch, minlength=num_graphs).astype(np.float32)
    per_core = []
    for c in range(n_cores):
        n0, n1 = int(bounds[c]), int(bounds[c + 1])
        xT = np.zeros((in_feat, n_own), np.float16)
        xT[:, : n1 - n0] = x[n0:n1].T.astype(np.float16)
        # gmask [128, NT*GPC]: node tile t, graph col g -> (batch==g)/graphsize
        gm = np.zeros((n_own, gpc), np.float16)
        loc = np.arange(n1 - n0)
        grel = (batch[n0:n1] - c * gpc).astype(np.int64)
        gs = gsizes[batch[n0:n1]]
        val = np.zeros(n1 - n0, np.float32)
        val[gs > 0] = 1.0 / gs[gs > 0]
        gm[loc, grel] = val.astype(np.float16)
        gmask = np.ascontiguousarray(
            gm.reshape(nt, P, gpc).transpose(1, 0, 2).reshape(P, nt * gpc))

        per_core.append(dict(
            xT=xT,
            eidx=to_i16(idx_arr[c]),
            edstrel=np.ascontiguousarray(dstrel_arr[c]),
            ewg=np.ascontiguousarray(w_arr[c]),
            gmask=gmask,
        ))

    return cfg, per_core


def make_in_maps(cfg, per_core, weights):
    wmap = {}
    for k in ("Wl1", "Wr1", "Wl2", "Wr2", "Wl3", "Wr3", "Wlin"):
        wmap[k] = np.ascontiguousarray(weights[k].astype(np.float16))
    for k in ("bl1", "bl2", "bl3", "blin"):
        wmap[k] = np.ascontiguousarray(weights[k].astype(np.float32).reshape(-1, 1))
    in_maps = []
    for c in range(cfg.n_cores):
        m = dict(per_core[c])
        m.update(wmap)
        in_maps.append(m)
    return in_maps


_PROGRAM_CACHE = {}


def kernel(x, edge_index, batch,
           Wl1, bl1, Wr1, Wl2, bl2, Wr2, Wl3, bl3, Wr3, Wlin, blin):
    x = np.asarray(x)
    cfg, per_core = preprocess(np.asarray(x, np.float32),
                               np.asarray(edge_index), np.asarray(batch))
    weights = dict(Wl1=np.asarray(Wl1), bl1=np.asarray(bl1), Wr1=np.asarray(Wr1),
                   Wl2=np.asarray(Wl2), bl2=np.asarray(bl2), Wr2=np.asarray(Wr2),
                   Wl3=np.asarray(Wl3), bl3=np.asarray(bl3), Wr3=np.asarray(Wr3),
                   Wlin=np.asarray(Wlin), blin=np.asarray(blin))
    in_maps = make_in_maps(cfg, per_core, weights)

    key = (cfg.n_own, cfg.sub_max, cfg.in_feat, cfg.num_graphs)
    if key not in _PROGRAM_CACHE:
        _PROGRAM_CACHE[key] = build_program(cfg)
    nc = _PROGRAM_CACHE[key]

    res = bass_utils.run_bass_kernel_spmd(
        nc, in_maps, core_ids=list(range(cfg.n_cores)),
    )
    out = np.empty((cfg.num_graphs, cfg.num_classes), np.float32)
    for c in range(cfg.n_cores):
        out[c * cfg.gpc:(c + 1) * cfg.gpc, :] = res.results[c]["out"].T
    return out
